# revision 28
# baseline (speedup 1.0000x reference)
"""2-layer GAT (GATConv x2 + log_softmax) on 8 Trainium2 NeuronCores.

Strategy (dst-sharded message passing):
  - Nodes are sharded contiguously across 8 cores (6250 each); every edge is
    owned by the core owning its dst node.  Edges are grouped by dst tile
    (128 dst nodes), split into A/B halves by src id (so gather indices fit
    int16), padded to 128-edge blocks with a cross-core-uniform schedule so
    all 8 cores run one SPMD program.
  - Layer-1 node phase is replicated: every core computes h = x@W1 (bf16,
    fp32 accum) for ALL nodes and writes a gather table
    [h(512) | a_src(8) | a_dst(8) | pad] bf16 per node.
  - Edge phase per 128-edge block: dma_gather rows by src, build a 0/1
    selection matrix SelT[e,d] = (dst_local[e] == d) on DVE, per-head
    weight multiply, then PE matmul SelT.T @ M accumulates the segment sum
    (and the softmax denominator) in PSUM per dst tile.
  - Scores: exp(leaky_relu(a_src[src] + a_dst[dst])) with a_dst gathered
    from a per-core table; softmax normalization is applied per dst tile
    after aggregation (alpha = w/denom pulled out of the edge sum).
  - Layer 2 (1 head, 16 ch) reuses the same block structure; the small
    z-table is exchanged with an AllGather collective.
"""
import os
import math
import numpy as np
import ml_dtypes

import concourse.bass as bass
import concourse.mybir as mybir
import concourse.tile as tile
import concourse.bacc as bacc
from concourse.masks import make_identity
from concourse.library_config import mlp

BF = ml_dtypes.bfloat16
dt = mybir.dt
AF = mybir.ActivationFunctionType
ALU = mybir.AluOpType

P = 128
ROW1 = 640     # table1 cols (bf16): [h 512 | a_src 8 | a_dst 8 | pad]
ROWZ = 128     # z-table cols (bf16): [z 16 | a_src2 1 | a_dst2 1 | pad]
BLKCAP = 22    # max blocks per gather group


# ----------------------------------------------------------------------------
# host-side schedule construction
# ----------------------------------------------------------------------------

def _wrap_idx(vals, slots):
    """Pad `vals` with 0 to `slots`, wrap into [128, slots/16] int16 layout."""
    v = np.zeros(slots, np.int64)
    v[: len(vals)] = vals
    a = v.reshape(-1, 16).T  # [16, slots/16]
    return np.tile(a, (8, 1)).astype(np.int16)


class Schedule:
    """Cross-core-uniform block schedule + per-core index arrays."""

    def __init__(self, src, dst, n_nodes, n_cores, force_split=None):
        self.n_nodes = n_nodes
        self.n_cores = n_cores
        self.npc = n_nodes // n_cores                 # real nodes per core
        self.nt = (self.npc + P - 1) // P             # dst tiles per core
        self.npcp = self.nt * P                       # padded nodes per core
        self.ntot_p = ((n_nodes + P - 1) // P) * P if n_cores == 1 else None
        # padded global table rows (node-id indexed)
        self.table_rows = ((n_nodes + P - 1) // P) * P
        self.table_rows = max(self.table_rows, self.npcp * n_cores)
        self.zrows = self.npcp * n_cores              # z-table rows (zid indexed)

        # split for int16 gathers: src <= SPLIT1-1 -> table A half;
        # zid(src) <= 32767 must also hold.
        if force_split is not None:
            self.split1 = force_split
        elif self.table_rows <= 32768 and self.zrows <= 32768:
            self.split1 = self.table_rows  # no B half
        else:
            # largest s with s-1 <= 32767 and zid(s-1) <= 32767
            s = min(32768, self.n_nodes)
            while s > 0:
                n = s - 1
                zid = (n // self.npc) * self.npcp + (n % self.npc)
                if zid <= 32767:
                    break
                s -= 1
            self.split1 = s
        self.zsplit = ((self.split1 - 1) // self.npc) * self.npcp + (
            (self.split1 - 1) % self.npc
        ) + 1 if self.split1 < self.table_rows else self.zrows

        core = dst // self.npc
        loc = dst - core * self.npc
        t = loc // P
        dloc = loc % P
        isB = src >= self.split1

        nc_, nt_ = n_cores, self.nt
        # counts[core, tile, {A,B}]
        key = (core * nt_ + t) * 2 + isB
        cnt = np.bincount(key, minlength=nc_ * nt_ * 2).reshape(nc_, nt_, 2)
        mx = cnt.max(axis=0)                            # [nt, 2]
        self.Ablk = np.ceil(mx[:, 0] / P).astype(int)
        self.Bblk = np.ceil(mx[:, 1] / P).astype(int)
        self.TBlk = self.Ablk + self.Bblk

        # groups: consecutive tiles, sum(TBlk) <= BLKCAP
        self.groups = []
        cur, acc = [], 0
        for ti in range(nt_):
            tb = int(self.TBlk[ti])
            if cur and acc + tb > BLKCAP:
                self.groups.append(cur)
                cur, acc = [], 0
            cur.append(ti)
            acc += tb
        if cur:
            self.groups.append(cur)

        # canonical block order & per-tile positions within group buffers
        # group buffer layout: [A-blocks of each tile in order, then B-blocks]
        self.g_ablk = []   # per group: total A blocks
        self.g_tblk = []   # per group: total blocks
        self.tile_apos = {}  # tile -> in-group A block offset
        self.tile_bpos = {}  # tile -> in-group block offset of its B blocks
        self.g_base = []     # per group: global block offset
        nblocks = 0
        for g, tl in enumerate(self.groups):
            ga = int(sum(self.Ablk[ti] for ti in tl))
            gt = int(sum(self.TBlk[ti] for ti in tl))
            self.g_ablk.append(ga)
            self.g_tblk.append(gt)
            ao = 0
            bo = ga
            for ti in tl:
                self.tile_apos[ti] = ao
                self.tile_bpos[ti] = bo
                ao += int(self.Ablk[ti])
                bo += int(self.Bblk[ti])
            self.g_base.append(nblocks)
            nblocks += gt
        self.nblocks = nblocks

        self.maxtb = int(self.TBlk.max())

        # per-core arrays
        # order edges by (core, tile, isB) stably
        order = np.lexsort((isB, t, core))
        self.per_core = []
        for c in range(nc_):
            m0 = order[core[order] == c]
            idx1A_cols, idx1B_cols, idx2A_cols, idx2B_cols = [], [], [], []
            dstloc = np.full((P, nblocks), 999.0, np.float32)
            for g, tl in enumerate(self.groups):
                a_src_l, b_src_l = [], []
                dl_A, dl_B = [], []
                for ti in tl:
                    e = m0[t[m0] == ti]
                    eA = e[~isB[e]]
                    eB = e[isB[e]]
                    nA = int(self.Ablk[ti]) * P
                    nB = int(self.Bblk[ti]) * P
                    sA = np.zeros(nA, np.int64)
                    sA[: len(eA)] = src[eA]
                    sB = np.zeros(nB, np.int64)
                    sB[: len(eB)] = src[eB] - self.split1
                    lA = np.full(nA, 999.0, np.float32)
                    lA[: len(eA)] = dloc[eA]
                    lB = np.full(nB, 999.0, np.float32)
                    lB[: len(eB)] = dloc[eB]
                    a_src_l.append(sA)
                    b_src_l.append(sB)
                    dl_A.append(lA)
                    dl_B.append(lB)
                gsA = np.concatenate(a_src_l) if a_src_l else np.zeros(0, np.int64)
                gsB = np.concatenate(b_src_l) if b_src_l else np.zeros(0, np.int64)
                gdl = np.concatenate(dl_A + dl_B) if (dl_A or dl_B) else np.zeros(0, np.float32)
                # L2 indices: zid mapping of global src
                def zid_of(v):
                    vv = np.asarray(v, np.int64)
                    return (vv // self.npc) * self.npcp + (vv % self.npc)
                g2A = zid_of(gsA)                       # gsA holds global src (pads=0)
                g2B = zid_of(gsB + self.split1) - self.zsplit
                idx1A_cols.append(_wrap_idx(gsA, len(gsA)))
                idx1B_cols.append(_wrap_idx(gsB, len(gsB)))
                idx2A_cols.append(_wrap_idx(g2A, len(g2A)))
                idx2B_cols.append(_wrap_idx(g2B, len(g2B)))
                gb = self.g_base[g]
                dstloc[:, gb : gb + self.g_tblk[g]] = gdl.reshape(-1, P).T
            cat = lambda ls: (
                np.concatenate(ls, axis=1) if ls and sum(x.shape[1] for x in ls) else np.zeros((P, 1), np.int16)
            )
            # partition-replicated transposed dstloc (pad -> 512, exact in bf16)
            dl = dstloc.T.copy()                       # [nblocks, P(edge)]
            dl[dl == 999.0] = 512.0
            dstlocT = np.broadcast_to(
                dl[None, :, :], (P, nblocks, P)
            ).astype(BF).copy()
            selSm = (dstloc[:, :, None] == np.arange(P, dtype=np.float32)
                     ).astype(BF)
            self.per_core.append(
                dict(
                    idx1A=cat(idx1A_cols), idx1B=cat(idx1B_cols),
                    idx2A=cat(idx2A_cols), idx2B=cat(idx2B_cols),
                    dstloc=dstloc, dstlocT=dstlocT, selSm=selSm,
                )
            )
        # column offsets per group in the concatenated idx arrays
        self.gA_coloff, self.gB_coloff = [], []
        a = b = 0
        for g in range(len(self.groups)):
            self.gA_coloff.append(a)
            self.gB_coloff.append(b)
            a += (self.g_ablk[g] * P) // 16
            b += ((self.g_tblk[g] - self.g_ablk[g]) * P) // 16
        self.totA_cols = max(a, 1)
        self.totB_cols = max(b, 1)


# ----------------------------------------------------------------------------
# device program
# ----------------------------------------------------------------------------

def build_program(sched: Schedule, n_cores: int, phase: str = 'full'):
    """Build the SPMD Bass/Tile program for the given schedule."""
    nc = bacc.Bacc(None, target_bir_lowering=False, debug=True, num_devices=n_cores,
                   num_swdge_queues=4)

    TR = sched.table_rows
    ZR = sched.zrows
    NT = sched.nt
    NPC, NPCP = sched.npc, sched.npcp
    NODE_TILES = TR // P

    # ---- inputs -------------------------------------------------------------
    xT = nc.dram_tensor("xT", [P, 2, TR], dt.float8e4, kind="ExternalInput")
    # W1 with interleaved out-cols (c*8+h) + fused [Asrc|Adst] cols -> 528
    W1r = nc.dram_tensor("W1r", [P, 2, 528], dt.float8e4, kind="ExternalInput")
    # wz = [W2p | W2p @ [as2|ad2]] with rows in interleaved order: [512, 18]
    wzr = nc.dram_tensor("wzr", [P, 4, 18], dt.bfloat16, kind="ExternalInput")
    idx1A = nc.dram_tensor("idx1A", [P, sched.totA_cols], dt.int16, kind="ExternalInput")
    idx1B = nc.dram_tensor("idx1B", [P, sched.totB_cols], dt.int16, kind="ExternalInput")
    idx2A = nc.dram_tensor("idx2A", [P, sched.totA_cols], dt.int16, kind="ExternalInput")
    idx2B = nc.dram_tensor("idx2B", [P, sched.totB_cols], dt.int16, kind="ExternalInput")
    dstlocr = nc.dram_tensor("dstloc", [P, sched.nblocks], dt.float32, kind="ExternalInput")
    dstlocTr = nc.dram_tensor("dstlocT", [P, sched.nblocks, P], dt.bfloat16, kind="ExternalInput")
    selSr = nc.dram_tensor("selSm", [P, sched.nblocks, P], dt.bfloat16, kind="ExternalInput")
    out_shard = nc.dram_tensor("out_shard", [NPCP, 16], dt.float32, kind="ExternalOutput")

    with tile.TileContext(nc) as tc:
        nc.gpsimd.load_library(mlp)
        with (
            tc.tile_pool(name="dram", bufs=1, space="DRAM") as dram,
            tc.tile_pool(name="const", bufs=1) as cpool,
        ):
            table1 = dram.tile([TR, ROW1], dt.bfloat16)
            adst_own = dram.tile([NPCP, 8], dt.bfloat16)
            cc_in = dram.tile([NPCP, ROWZ], dt.bfloat16)
            cc_out = dram.tile([ZR, ROWZ], dt.bfloat16,
                               addr_space=("Shared" if n_cores > 1 else "Local"))

            # ---- constants -------------------------------------------------
            iota_i = cpool.tile([P, P], dt.int32)
            nc.gpsimd.iota(iota_i[:], pattern=[[1, P]], base=0, channel_multiplier=0)
            iota_bf = cpool.tile([P, P], dt.bfloat16)
            nc.vector.tensor_copy(iota_bf[:], iota_i[:])
            iota_ci = cpool.tile([P, 1], dt.int32)
            nc.gpsimd.iota(iota_ci[:], pattern=[[0, 1]], base=0, channel_multiplier=1)
            iota_cf = cpool.tile([P, 1], dt.float32)
            nc.vector.tensor_copy(iota_cf[:], iota_ci[:])
            ident = cpool.tile([P, P], dt.bfloat16)
            make_identity(nc, ident[:])

            W1s = cpool.tile([P, 2, 528], dt.float8e4)
            nc.sync.dma_start(W1s[:], W1r[:])
            wz = cpool.tile([P, 4, 18], dt.bfloat16)
            nc.sync.dma_start(wz[:], wzr[:])
            dstloc_s = cpool.tile([P, sched.nblocks], dt.float32)
            nc.sync.dma_start(dstloc_s[:], dstlocr[:])
            i1A = cpool.tile([P, sched.totA_cols], dt.int16)
            nc.sync.dma_start(i1A[:], idx1A[:])
            i1B = cpool.tile([P, sched.totB_cols], dt.int16)
            nc.sync.dma_start(i1B[:], idx1B[:])
            i2A = cpool.tile([P, sched.totA_cols], dt.int16)
            nc.sync.dma_start(i2A[:], idx2A[:])
            i2B = cpool.tile([P, sched.totB_cols], dt.int16)
            nc.sync.dma_start(i2B[:], idx2B[:])
            adst_all = cpool.tile([P, NT, 8], dt.bfloat16)
            azdst_all = cpool.tile([P, NT, 1], dt.bfloat16)
            o2_all = cpool.tile([P, NT, 16], dt.float32)   # deferred L2 softmax

            # ---- P1: replicated node phase --------------------------------
            XB = 8  # node tiles per x load / per table write
            with (
                tc.tile_pool(name="p1sb", bufs=3) as p1sb,
                tc.tile_pool(name="p1ps", bufs=3, space="PSUM") as p1ps,
            ):
                DR = mybir.MatmulPerfMode.DoubleRow
                for tq in range(0, NODE_TILES, XB):
                    nb = min(XB, NODE_TILES - tq)
                    xt = p1sb.tile([P, 2, nb * P], dt.float8e4, tag="xt")
                    nc.sync.dma_start(xt[:], xT[:, :, tq * P: tq * P + nb * P])
                    rowt = p1sb.tile([P, XB, ROW1], dt.bfloat16, tag="rowt")
                    for u in range(nb):
                        ph = p1ps.tile([P, 512], dt.float32, tag="ph")
                        pa = p1ps.tile([P, 16], dt.float32, tag="pa")
                        lhs = xt[:, :, u * P:(u + 1) * P]
                        nc.tensor.matmul(ph[:], lhs, W1s[:, :, 0:512],
                                         perf_mode=DR, start=True, stop=True)
                        nc.tensor.matmul(pa[:], lhs, W1s[:, :, 512:528],
                                         perf_mode=DR, start=True, stop=True)
                        if u % 2 == 0:
                            nc.scalar.activation(rowt[:, u, 0:512], ph[:],
                                                 AF.Copy, scale=0.0625)
                        else:
                            nc.vector.tensor_scalar(
                                out=rowt[:, u, 0:512], in0=ph[:], scalar1=0.0625,
                                scalar2=None, op0=ALU.mult,
                            )
                        nc.vector.tensor_scalar(
                            out=rowt[:, u, 512:528], in0=pa[:], scalar1=0.0625,
                            scalar2=None, op0=ALU.mult,
                        )
                    # one batched table write per XB tiles (4x fewer sync
                    # queue DMA triggers)
                    nc.sync.dma_start(
                        table1[tq * P:(tq + nb) * P, :].rearrange(
                            "(u p) r -> p u r", p=P),
                        rowt[:, 0:nb, :],
                    )

            # ---- P1.5: per-core a_dst table (SBUF, tile-major) ------------
            if phase not in ("p1",):
                rbase = nc.sync.partition_id() * NPC
                nc.sync.dma_start(
                    adst_own[:, :],
                    table1[bass.ds(rbase, NPCP), 520:528],
                )
                nc.sync.dma_start(
                    adst_all[:],
                    adst_own[:].rearrange("(t p) c -> p t c", p=P),
                )

            # ---- edge phase helper ----------------------------------------
            def edge_phase(layer):
                """layer 1: table1 gathers, 8 heads; layer 2: z-table, 1 head."""
                sub = os.environ.get("GAT_L1SUB", "full")
                if layer == 1:
                    g_src_tab_A, g_src_tab_B = table1, table1[sched.split1:, :]
                    g_elem, g_row = ROW1, ROW1
                    iA, iB = i1A, i1B
                    NH = 8
                    adst_t = adst_all
                else:
                    g_src_tab_A, g_src_tab_B = cc_out, cc_out[sched.zsplit:, :]
                    g_elem, g_row = ROWZ, ROWZ
                    iA, iB = i2A, i2B
                    NH = 1
                    adst_t = azdst_all
                MAXTB = sched.maxtb
                edeep = 3 if layer == 1 else 4   # keep the gather queue fed
                with (
                    tc.tile_pool(name=f"ed{layer}", bufs=edeep) as ep,
                    tc.tile_pool(name=f"dl{layer}", bufs=2 if layer == 1 else 4) as dp,
                    tc.tile_pool(name=f"sl{layer}", bufs=2 if layer == 1 else 3) as sp,
                    tc.tile_pool(name=f"ms{layer}", bufs=2 if layer == 1 else 3) as mp,
                    tc.tile_pool(name=f"eb{layer}", bufs=3 if layer == 1 else 4) as bp,
                    tc.tile_pool(name=f"os{layer}", bufs=8) as op_,
                    tc.tile_pool(name=f"ep{layer}", bufs=2, space="PSUM") as pp,
                    tc.tile_pool(name=f"eo{layer}", bufs=2, space="PSUM") as po,
                    tc.tile_pool(name=f"eq{layer}", bufs=2, space="PSUM") as pq,
                    tc.tile_pool(name=f"ez{layer}", bufs=1, space="PSUM") as pz1,
                ):
                    pending = []
                    for g, tl in enumerate(sched.groups):
                        GB = sched.g_tblk[g]
                        GA = sched.g_ablk[g]
                        nA, nB_ = GA * P, (GB - GA) * P
                        gb = sched.g_base[g]
                        hg = ep.tile([P, GB, g_elem], dt.bfloat16, tag="hg")
                        if nA:
                            nc.gpsimd.dma_gather(
                                hg[:, 0:GA, :], g_src_tab_A[:],
                                iA[:, sched.gA_coloff[g]: sched.gA_coloff[g] + nA // 16],
                                nA, nA, g_elem, single_packet=False,
                                queue_num=(2 * g) % 4,
                            )
                        if nB_:
                            nc.gpsimd.dma_gather(
                                hg[:, GA:GB, :], g_src_tab_B,
                                iB[:, sched.gB_coloff[g]: sched.gB_coloff[g] + nB_ // 16],
                                nB_, nB_, g_elem, elem_step=g_row,
                                single_packet=False,
                                queue_num=(2 * g + 1) % 4,
                            )
                        # transposed dstloc rows for this group (HWDGE stream)
                        dlT = dp.tile([P, GB, P], dt.bfloat16, tag="dlT")
                        nc.sync.dma_start(dlT[:], dstlocTr[:, gb:gb + GB, :])
                        if sub == "gather":
                            continue
                        # per-tile span lists: (in-group block offset, count)
                        tspans = []
                        for ti in tl:
                            ab, bb = int(sched.Ablk[ti]), int(sched.Bblk[ti])
                            if ab + bb == 0:
                                continue
                            spans = []
                            if ab:
                                spans.append((sched.tile_apos[ti], ab))
                            if bb:
                                spans.append((sched.tile_bpos[ti], bb))
                            tspans.append((ti, spans, ab + bb))
                        # ---- stage 1: selection matrices for whole group ----
                        # selS streamed from DRAM (host-precomputed 0/1
                        # matrix; frees the DVE); sel2S built at 2x on DVE.
                        selG = sp.tile([P, GB, P], dt.bfloat16, tag="selS")
                        nc.sync.dma_start(selG[:], selSr[:, gb:gb + GB, :])
                        sel2G = sp.tile([P, GB, P], dt.bfloat16, tag="sel2S")
                        nc.vector.tensor_scalar(
                            out=sel2G[:], in0=dlT[:], scalar1=iota_cf[:, 0:1],
                            scalar2=None, op0=ALU.is_equal,
                        )
                        # ---- stage 2: a_dst gathers on PE ----
                        aDsG = pq.tile([P, GB, NH], dt.float32, tag="aDs")
                        for ti, spans, tb in tspans:
                            for (o, n) in spans:
                                for k in range(n):
                                    nc.tensor.matmul(
                                        aDsG[:, o + k, :], sel2G[:, o + k, :],
                                        adst_t[:, ti, :], start=True, stop=True,
                                    )
                        # ---- stage 3: scores for whole group ----
                        if layer == 1:
                            wG = bp.tile([P, GB * 8], dt.bfloat16, tag="wbf")
                        else:
                            wG = bp.tile([P, GB * 1], dt.float32, tag="wt")
                        stG = bp.tile([P, GB * NH], dt.float32, tag="st")
                        lkG = bp.tile([P, GB * NH], dt.float32, tag="lk")
                        a_s = hg[:, :, 512:520] if layer == 1 else hg[:, :, 17:18]
                        nc.vector.tensor_tensor(
                            out=stG[:].rearrange("p (n k) -> p n k", k=NH),
                            in0=a_s, in1=aDsG[:], op=ALU.add,
                        )
                        nc.vector.scalar_tensor_tensor(
                            lkG[:], stG[:], 0.2, stG[:], ALU.mult, ALU.max
                        )
                        nc.scalar.activation(wG[:], lkG[:], AF.Exp)
                        # ---- flush deferred closes of the previous group ----
                        if sub != "blocks":
                            for args in pending:
                                close_tile(*args)
                            pending.clear()
                        if sub == "score":
                            continue
                        # ---- stage 4: Ms builds for whole group ----
                        MsG = mp.tile(
                            [P, GB, 512 if layer == 1 else 17],
                            dt.bfloat16, tag="Ms",
                        )
                        if layer == 1:
                            # h cols interleaved (c*8+h): w broadcast is
                            # middle-dim -> last dim stays packed -> DVE 2x.
                            wbc = wG[:].rearrange(
                                "p (n a b) -> p n a b", n=GB, a=1
                            ).to_broadcast([P, GB, 64, 8])
                            nc.vector.tensor_tensor(
                                out=MsG[:].rearrange("p n (a b) -> p n a b", b=8),
                                in0=hg[:, :, 0:512].rearrange(
                                    "p n (a b) -> p n a b", b=8),
                                in1=wbc,
                                op=ALU.mult,
                            )
                        else:
                            w2bc = wG[:].rearrange(
                                "p (n b) -> p n b", b=1).to_broadcast([P, GB, 17])
                            nc.vector.tensor_tensor(
                                out=MsG[:], in0=hg[:, :, 0:17], in1=w2bc,
                                op=ALU.mult,
                            )
                        # ---- stage 5: aggregation matmuls + psum flush ----
                        for ti, spans, tb in tspans:
                            if layer == 1:
                                psum_o = pp.tile([P, 512], dt.float32, tag="psO")
                                psum_d = po.tile([P, 8], dt.float32, tag="psD")
                            else:
                                psum_o = pp.tile([P, 17], dt.float32, tag="psO")
                                psum_d = None
                            j = 0
                            for (o, n) in spans:
                                for k in range(n):
                                    jj = o + k
                                    first, last = (j == 0), (j == tb - 1)
                                    nc.tensor.matmul(psum_o[:], selG[:, jj, :],
                                                     MsG[:, jj, :],
                                                     start=first, stop=last)
                                    if layer == 1:
                                        nc.tensor.matmul(
                                            psum_d[:], selG[:, jj, :],
                                            wG[:, jj * 8:jj * 8 + 8],
                                            start=first, stop=last,
                                        )
                                    j += 1
                            # flush psums to SBUF right away (scalar engine):
                            # frees the banks and closes read fast SBUF tiles
                            if layer == 1:
                                oS = op_.tile([P, 512], dt.bfloat16, tag="oS")
                                nc.scalar.copy(oS[:], psum_o[:])
                                dS = op_.tile([P, 8], dt.float32, tag="dS")
                                nc.scalar.copy(dS[:], psum_d[:])
                            else:
                                oS = op_.tile([P, 17], dt.float32, tag="oS")
                                nc.scalar.copy(oS[:], psum_o[:])
                                dS = None
                            pending.append((layer, ti, oS, dS, bp, pz1, pz1))
                    # drain the last group's closes
                    if sub not in ("gather", "score", "blocks"):
                        for args in pending:
                            close_tile(*args)
                        pending.clear()

            # ---- tile close -------------------------------------------------
            def close_tile(layer, ti, oS, dS, bp, ptp, pzp):
                if layer == 1:
                    r = bp.tile([P, 8], dt.float32, tag="r")
                    nc.vector.reciprocal(r[:], dS[:])
                    o1 = bp.tile([P, 512], dt.bfloat16, tag="o1")
                    o13 = o1[:].rearrange("p (c h) -> p c h", h=8)
                    rbc = r[:].rearrange("p (a h) -> p a h", a=1).to_broadcast([P, 64, 8])
                    nc.vector.tensor_tensor(
                        out=o13,
                        in0=oS[:].rearrange("p (c h) -> p c h", h=8),
                        in1=rbc, op=ALU.mult,
                    )
                    # elu: h2 = max(o1,0) + exp(min(o1,0)) - 1
                    u = bp.tile([P, 512], dt.bfloat16, tag="u")
                    nc.vector.tensor_scalar_min(u[:], o1[:], 0.0)
                    e1 = bp.tile([P, 512], dt.bfloat16, tag="e1")
                    nc.scalar.activation(e1[:], u[:], AF.Exp)
                    rv = bp.tile([P, 512], dt.bfloat16, tag="rv")
                    nc.vector.tensor_scalar_max(rv[:], o1[:], 0.0)
                    h2 = bp.tile([P, 512], dt.bfloat16, tag="h2")
                    nc.vector.scalar_tensor_tensor(
                        h2[:], e1[:], -1.0, rv[:], ALU.add, ALU.add
                    )
                    # transpose h2 -> z matmuls
                    pz = pzp.tile([P, 18], dt.float32, tag="psZ")
                    for c in range(4):
                        ptr = ptp.tile([P, P], dt.bfloat16, tag="psT")
                        nc.tensor.transpose(ptr[:], h2[:, c * P:(c + 1) * P], ident[:])
                        h2T = bp.tile([P, P], dt.bfloat16, tag="h2T")
                        nc.scalar.copy(h2T[:], ptr[:])
                        nc.tensor.matmul(pz[:], h2T[:], wz[:, c, :], start=(c == 0), stop=(c == 3))
                    # z-row layout: [z 16 | 1.0 | a_src2 | a_dst2 | junk]
                    zrow = bp.tile([P, ROWZ], dt.bfloat16, tag="zrow")
                    nc.vector.tensor_copy(zrow[:, 0:16], pz[:, 0:16])
                    nc.vector.memset(zrow[:, 16:17], 1.0)
                    nc.vector.tensor_copy(zrow[:, 17:19], pz[:, 16:18])
                    nc.sync.dma_start(cc_in[ti * P:(ti + 1) * P, :], zrow[:])
                else:
                    # defer log_softmax to one batched pass (avoids Exp<->Ln
                    # activation-table thrash against the edge-phase Exp)
                    r2 = bp.tile([P, 1], dt.float32, tag="r2")
                    nc.vector.reciprocal(r2[:], oS[:, 16:17])
                    nc.vector.tensor_scalar_mul(
                        o2_all[:, ti, :], oS[:, 0:16], r2[:, 0:1])

            if phase not in ("p1", "p15"):
                edge_phase(1)

            if phase in ("cc", "full"):
                # ---- z-table exchange -------------------------------------
                if n_cores == 1:
                    nc.sync.dma_start(cc_out[:, :], cc_in[:, :])
                else:
                    nc.gpsimd.collective_compute(
                        "AllGather", ALU.bypass,
                        ins=[cc_in[:]], outs=[cc_out[:]],
                        replica_groups=[list(range(n_cores))],
                    )

            if phase == "full":
                nc.sync.dma_start(
                    azdst_all[:],
                    cc_in[:, 18:19].rearrange("(t p) c -> p t c", p=P),
                )
                edge_phase(2)
                # batched log_softmax over all dst tiles (2 act-table loads)
                with tc.tile_pool(name="fin", bufs=1) as fp:
                    mx = fp.tile([P, NT, 1], dt.float32)
                    nc.vector.tensor_reduce(
                        mx[:], o2_all[:], axis=mybir.AxisListType.X, op=ALU.max)
                    o2m = fp.tile([P, NT, 16], dt.float32)
                    nc.vector.tensor_tensor(
                        out=o2m[:], in0=o2_all[:],
                        in1=mx[:].to_broadcast([P, NT, 16]), op=ALU.subtract)
                    ex = fp.tile([P, NT, 16], dt.float32)
                    nc.scalar.activation(ex[:], o2m[:], AF.Exp)
                    ssum = fp.tile([P, NT, 1], dt.float32)
                    nc.vector.tensor_reduce(
                        ssum[:], ex[:], axis=mybir.AxisListType.X, op=ALU.add)
                    lse = fp.tile([P, NT, 1], dt.float32)
                    nc.scalar.activation(lse[:], ssum[:], AF.Ln)
                    res = fp.tile([P, NT, 16], dt.float32)
                    nc.vector.tensor_tensor(
                        out=res[:], in0=o2m[:],
                        in1=lse[:].to_broadcast([P, NT, 16]), op=ALU.subtract)
                    nc.sync.dma_start(
                        out_shard[:].rearrange("(t p) c -> p t c", p=P), res[:])

    nc.compile()
    return nc


# ----------------------------------------------------------------------------
# host entry
# ----------------------------------------------------------------------------

def _blockdiag(att, heads, hid):
    """[heads, hid] -> [heads*hid, heads] block diagonal."""
    out = np.zeros((heads * hid, max(heads, 1)), np.float32)
    for h in range(heads):
        out[h * hid:(h + 1) * hid, h] = att[h]
    return out


def prepare_inputs(inputs, sched: Schedule):
    x = np.asarray(inputs["x"], np.float32)
    ei = np.asarray(inputs["edge_index"])
    W1 = np.asarray(inputs["W1"], np.float32)
    as1 = np.asarray(inputs["att_src1"], np.float32)
    ad1 = np.asarray(inputs["att_dst1"], np.float32)
    W2 = np.asarray(inputs["W2"], np.float32)
    as2 = np.asarray(inputs["att_src2"], np.float32)
    ad2 = np.asarray(inputs["att_dst2"], np.float32)

    N, IN = x.shape
    TR = sched.table_rows
    xp = np.zeros((TR, IN), np.float32)
    xp[:N] = x
    F8 = ml_dtypes.float8_e4m3
    xTb = np.ascontiguousarray(
        xp.T.reshape(2, P, TR).transpose(1, 0, 2)).astype(F8)
    # interleave h columns: new col j = c*8 + h  <->  old col h*64 + c
    perm = np.array([(j % 8) * 64 + (j // 8) for j in range(512)], np.int64)
    acat = np.concatenate(
        [_blockdiag(as1, 8, 64), _blockdiag(ad1, 8, 64)], axis=1)  # [512, 16]
    wcat = W1 @ acat                                     # [256, 16]
    W1c = np.concatenate([W1[:, perm], wcat], axis=1)    # [256, 528]
    W1b = np.ascontiguousarray(
        (W1c * 16.0).reshape(2, P, 528).transpose(1, 0, 2)).astype(F8)
    att2b = np.concatenate([as2.T, ad2.T], axis=1)       # [16, 2]
    wzf = np.concatenate([W2[perm, :], W2[perm, :] @ att2b], axis=1)  # [512, 18]
    wzb = np.ascontiguousarray(wzf.reshape(4, P, 18).transpose(1, 0, 2)).astype(BF)

    shared = dict(xT=xTb, W1r=W1b, wzr=wzb)
    maps = []
    for c in range(sched.n_cores):
        pc = sched.per_core[c]
        m = dict(shared)
        m.update(
            idx1A=pc["idx1A"], idx1B=pc["idx1B"], idx2A=pc["idx2A"],
            idx2B=pc["idx2B"], dstloc=pc["dstloc"], dstlocT=pc["dstlocT"],
            selSm=pc["selSm"],
        )
        maps.append(m)
    return maps


_LAST_RESULT = {}


def kernel(**inputs):
    from concourse.bass_utils import run_bass_kernel_spmd

    x = np.asarray(inputs["x"], np.float32)
    ei = np.asarray(inputs["edge_index"], np.int64)
    N = x.shape[0]
    n_cores = 8
    loops = np.arange(N, dtype=np.int64)
    src = np.concatenate([ei[0], loops])
    dst = np.concatenate([ei[1], loops])

    sched = Schedule(src, dst, N, n_cores)
    phase = os.environ.get("GAT_PHASE", "full")
    nc = build_program(sched, n_cores, phase=phase)
    in_maps = prepare_inputs(inputs, sched)

    trace = bool(int(os.environ.get("GAT_TRACE", "0")))
    res = run_bass_kernel_spmd(
        nc, in_maps, core_ids=list(range(n_cores)), trace=trace,
    )
    _LAST_RESULT["res"] = res

    out = np.zeros((N, 16), np.float32)
    for c in range(n_cores):
        sh = res.results[c]["out_shard"]
        n0 = c * sched.npc
        out[n0:n0 + sched.npc] = sh[: sched.npc]
    return out



# revision 29
# speedup vs baseline: 1.0600x; 1.0600x over previous
"""2-layer GAT (GATConv x2 + log_softmax) on 8 Trainium2 NeuronCores.

Strategy (dst-sharded message passing):
  - Nodes are sharded contiguously across 8 cores (6250 each); every edge is
    owned by the core owning its dst node.  Edges are grouped by dst tile
    (128 dst nodes), split into A/B halves by src id (so gather indices fit
    int16), padded to 128-edge blocks with a cross-core-uniform schedule so
    all 8 cores run one SPMD program.
  - Layer-1 node phase is replicated: every core computes h = x@W1 (bf16,
    fp32 accum) for ALL nodes and writes a gather table
    [h(512) | a_src(8) | a_dst(8) | pad] bf16 per node.
  - Edge phase per 128-edge block: dma_gather rows by src, build a 0/1
    selection matrix SelT[e,d] = (dst_local[e] == d) on DVE, per-head
    weight multiply, then PE matmul SelT.T @ M accumulates the segment sum
    (and the softmax denominator) in PSUM per dst tile.
  - Scores: exp(leaky_relu(a_src[src] + a_dst[dst])) with a_dst gathered
    from a per-core table; softmax normalization is applied per dst tile
    after aggregation (alpha = w/denom pulled out of the edge sum).
  - Layer 2 (1 head, 16 ch) reuses the same block structure; the small
    z-table is exchanged with an AllGather collective.
"""
import os
import math
import numpy as np
import ml_dtypes

import concourse.bass as bass
import concourse.mybir as mybir
import concourse.tile as tile
import concourse.bacc as bacc
from concourse.masks import make_identity
from concourse.library_config import mlp

BF = ml_dtypes.bfloat16
dt = mybir.dt
AF = mybir.ActivationFunctionType
ALU = mybir.AluOpType

P = 128
ROW1 = 640     # table1 cols (bf16): [h 512 | a_src 8 | a_dst 8 | pad]
ROWZ = 128     # z-table cols (bf16): [z 16 | a_src2 1 | a_dst2 1 | pad]
BLKCAP = 22    # max blocks per gather group


# ----------------------------------------------------------------------------
# host-side schedule construction
# ----------------------------------------------------------------------------

def _wrap_idx(vals, slots):
    """Pad `vals` with 0 to `slots`, wrap into [128, slots/16] int16 layout."""
    v = np.zeros(slots, np.int64)
    v[: len(vals)] = vals
    a = v.reshape(-1, 16).T  # [16, slots/16]
    return np.tile(a, (8, 1)).astype(np.int16)


class Schedule:
    """Cross-core-uniform block schedule + per-core index arrays."""

    def __init__(self, src, dst, n_nodes, n_cores, force_split=None):
        self.n_nodes = n_nodes
        self.n_cores = n_cores
        self.npc = n_nodes // n_cores                 # real nodes per core
        self.nt = (self.npc + P - 1) // P             # dst tiles per core
        self.npcp = self.nt * P                       # padded nodes per core
        self.ntot_p = ((n_nodes + P - 1) // P) * P if n_cores == 1 else None
        # padded global table rows (node-id indexed)
        self.table_rows = ((n_nodes + P - 1) // P) * P
        self.table_rows = max(self.table_rows, self.npcp * n_cores)
        self.zrows = self.npcp * n_cores              # z-table rows (zid indexed)

        # split for int16 gathers: src <= SPLIT1-1 -> table A half;
        # zid(src) <= 32767 must also hold.
        if force_split is not None:
            self.split1 = force_split
        elif self.table_rows <= 32768 and self.zrows <= 32768:
            self.split1 = self.table_rows  # no B half
        else:
            # largest s with s-1 <= 32767 and zid(s-1) <= 32767
            s = min(32768, self.n_nodes)
            while s > 0:
                n = s - 1
                zid = (n // self.npc) * self.npcp + (n % self.npc)
                if zid <= 32767:
                    break
                s -= 1
            self.split1 = s
        self.zsplit = ((self.split1 - 1) // self.npc) * self.npcp + (
            (self.split1 - 1) % self.npc
        ) + 1 if self.split1 < self.table_rows else self.zrows

        core = dst // self.npc
        loc = dst - core * self.npc
        t = loc // P
        dloc = loc % P
        isB = src >= self.split1

        nc_, nt_ = n_cores, self.nt
        # counts[core, tile, {A,B}]
        key = (core * nt_ + t) * 2 + isB
        cnt = np.bincount(key, minlength=nc_ * nt_ * 2).reshape(nc_, nt_, 2)
        mx = cnt.max(axis=0)                            # [nt, 2]
        self.Ablk = np.ceil(mx[:, 0] / P).astype(int)
        self.Bblk = np.ceil(mx[:, 1] / P).astype(int)
        self.TBlk = self.Ablk + self.Bblk

        # groups: consecutive tiles, sum(TBlk) <= BLKCAP
        self.groups = []
        cur, acc = [], 0
        for ti in range(nt_):
            tb = int(self.TBlk[ti])
            if cur and acc + tb > BLKCAP:
                self.groups.append(cur)
                cur, acc = [], 0
            cur.append(ti)
            acc += tb
        if cur:
            self.groups.append(cur)

        # canonical block order & per-tile positions within group buffers
        # group buffer layout: [A-blocks of each tile in order, then B-blocks]
        self.g_ablk = []   # per group: total A blocks
        self.g_tblk = []   # per group: total blocks
        self.tile_apos = {}  # tile -> in-group A block offset
        self.tile_bpos = {}  # tile -> in-group block offset of its B blocks
        self.g_base = []     # per group: global block offset
        nblocks = 0
        for g, tl in enumerate(self.groups):
            ga = int(sum(self.Ablk[ti] for ti in tl))
            gt = int(sum(self.TBlk[ti] for ti in tl))
            self.g_ablk.append(ga)
            self.g_tblk.append(gt)
            ao = 0
            bo = ga
            for ti in tl:
                self.tile_apos[ti] = ao
                self.tile_bpos[ti] = bo
                ao += int(self.Ablk[ti])
                bo += int(self.Bblk[ti])
            self.g_base.append(nblocks)
            nblocks += gt
        self.nblocks = nblocks

        self.maxtb = int(self.TBlk.max())

        # per-core arrays
        # order edges by (core, tile, isB) stably
        order = np.lexsort((isB, t, core))
        self.per_core = []
        for c in range(nc_):
            m0 = order[core[order] == c]
            idx1A_cols, idx1B_cols, idx2A_cols, idx2B_cols = [], [], [], []
            dstloc = np.full((P, nblocks), 999.0, np.float32)
            for g, tl in enumerate(self.groups):
                a_src_l, b_src_l = [], []
                dl_A, dl_B = [], []
                for ti in tl:
                    e = m0[t[m0] == ti]
                    eA = e[~isB[e]]
                    eB = e[isB[e]]
                    nA = int(self.Ablk[ti]) * P
                    nB = int(self.Bblk[ti]) * P
                    sA = np.zeros(nA, np.int64)
                    sA[: len(eA)] = src[eA]
                    sB = np.zeros(nB, np.int64)
                    sB[: len(eB)] = src[eB] - self.split1
                    lA = np.full(nA, 999.0, np.float32)
                    lA[: len(eA)] = dloc[eA]
                    lB = np.full(nB, 999.0, np.float32)
                    lB[: len(eB)] = dloc[eB]
                    a_src_l.append(sA)
                    b_src_l.append(sB)
                    dl_A.append(lA)
                    dl_B.append(lB)
                gsA = np.concatenate(a_src_l) if a_src_l else np.zeros(0, np.int64)
                gsB = np.concatenate(b_src_l) if b_src_l else np.zeros(0, np.int64)
                gdl = np.concatenate(dl_A + dl_B) if (dl_A or dl_B) else np.zeros(0, np.float32)
                # L2 indices: zid mapping of global src
                def zid_of(v):
                    vv = np.asarray(v, np.int64)
                    return (vv // self.npc) * self.npcp + (vv % self.npc)
                g2A = zid_of(gsA)                       # gsA holds global src (pads=0)
                g2B = zid_of(gsB + self.split1) - self.zsplit
                idx1A_cols.append(_wrap_idx(gsA, len(gsA)))
                idx1B_cols.append(_wrap_idx(gsB, len(gsB)))
                idx2A_cols.append(_wrap_idx(g2A, len(g2A)))
                idx2B_cols.append(_wrap_idx(g2B, len(g2B)))
                gb = self.g_base[g]
                dstloc[:, gb : gb + self.g_tblk[g]] = gdl.reshape(-1, P).T
            cat = lambda ls: (
                np.concatenate(ls, axis=1) if ls and sum(x.shape[1] for x in ls) else np.zeros((P, 1), np.int16)
            )
            # partition-replicated transposed dstloc (pad -> 512, exact in bf16)
            dl = dstloc.T.copy()                       # [nblocks, P(edge)]
            dl[dl == 999.0] = 512.0
            dstlocT = np.broadcast_to(
                dl[None, :, :], (P, nblocks, P)
            ).astype(BF).copy()
            self.per_core.append(
                dict(
                    idx1A=cat(idx1A_cols), idx1B=cat(idx1B_cols),
                    idx2A=cat(idx2A_cols), idx2B=cat(idx2B_cols),
                    dstloc=dstloc, dstlocT=dstlocT,
                )
            )
        # column offsets per group in the concatenated idx arrays
        self.gA_coloff, self.gB_coloff = [], []
        a = b = 0
        for g in range(len(self.groups)):
            self.gA_coloff.append(a)
            self.gB_coloff.append(b)
            a += (self.g_ablk[g] * P) // 16
            b += ((self.g_tblk[g] - self.g_ablk[g]) * P) // 16
        self.totA_cols = max(a, 1)
        self.totB_cols = max(b, 1)


# ----------------------------------------------------------------------------
# device program
# ----------------------------------------------------------------------------

def build_program(sched: Schedule, n_cores: int, phase: str = 'full'):
    """Build the SPMD Bass/Tile program for the given schedule."""
    nc = bacc.Bacc(None, target_bir_lowering=False, debug=True, num_devices=n_cores,
                   num_swdge_queues=4)

    TR = sched.table_rows
    ZR = sched.zrows
    NT = sched.nt
    NPC, NPCP = sched.npc, sched.npcp
    NODE_TILES = TR // P

    # ---- inputs -------------------------------------------------------------
    xT = nc.dram_tensor("xT", [P, 2, TR], dt.float8e4, kind="ExternalInput")
    # W1 with interleaved out-cols (c*8+h) + fused [Asrc|Adst] cols -> 528
    W1r = nc.dram_tensor("W1r", [P, 2, 528], dt.float8e4, kind="ExternalInput")
    # wz = [W2p | W2p @ [as2|ad2]] with rows in interleaved order: [512, 18]
    wzr = nc.dram_tensor("wzr", [P, 4, 18], dt.bfloat16, kind="ExternalInput")
    idx1A = nc.dram_tensor("idx1A", [P, sched.totA_cols], dt.int16, kind="ExternalInput")
    idx1B = nc.dram_tensor("idx1B", [P, sched.totB_cols], dt.int16, kind="ExternalInput")
    idx2A = nc.dram_tensor("idx2A", [P, sched.totA_cols], dt.int16, kind="ExternalInput")
    idx2B = nc.dram_tensor("idx2B", [P, sched.totB_cols], dt.int16, kind="ExternalInput")
    dstlocr = nc.dram_tensor("dstloc", [P, sched.nblocks], dt.float32, kind="ExternalInput")
    dstlocTr = nc.dram_tensor("dstlocT", [P, sched.nblocks, P], dt.bfloat16, kind="ExternalInput")
    out_shard = nc.dram_tensor("out_shard", [NPCP, 16], dt.float32, kind="ExternalOutput")

    with tile.TileContext(nc) as tc:
        nc.gpsimd.load_library(mlp)
        with (
            tc.tile_pool(name="dram", bufs=1, space="DRAM") as dram,
            tc.tile_pool(name="const", bufs=1) as cpool,
        ):
            table1 = dram.tile([TR, ROW1], dt.bfloat16)
            adst_own = dram.tile([NPCP, 8], dt.bfloat16)
            cc_in = dram.tile([NPCP, ROWZ], dt.bfloat16)
            cc_out = dram.tile([ZR, ROWZ], dt.bfloat16,
                               addr_space=("Shared" if n_cores > 1 else "Local"))

            # ---- constants -------------------------------------------------
            iota_i = cpool.tile([P, P], dt.int32)
            nc.gpsimd.iota(iota_i[:], pattern=[[1, P]], base=0, channel_multiplier=0)
            iota_bf = cpool.tile([P, P], dt.bfloat16)
            nc.vector.tensor_copy(iota_bf[:], iota_i[:])
            iota_ci = cpool.tile([P, 1], dt.int32)
            nc.gpsimd.iota(iota_ci[:], pattern=[[0, 1]], base=0, channel_multiplier=1)
            iota_cf = cpool.tile([P, 1], dt.float32)
            nc.vector.tensor_copy(iota_cf[:], iota_ci[:])
            ident = cpool.tile([P, P], dt.bfloat16)
            make_identity(nc, ident[:])

            W1s = cpool.tile([P, 2, 528], dt.float8e4)
            nc.sync.dma_start(W1s[:], W1r[:])
            wz = cpool.tile([P, 4, 18], dt.bfloat16)
            nc.sync.dma_start(wz[:], wzr[:])
            dstloc_s = cpool.tile([P, sched.nblocks], dt.float32)
            nc.sync.dma_start(dstloc_s[:], dstlocr[:])
            i1A = cpool.tile([P, sched.totA_cols], dt.int16)
            nc.sync.dma_start(i1A[:], idx1A[:])
            i1B = cpool.tile([P, sched.totB_cols], dt.int16)
            nc.sync.dma_start(i1B[:], idx1B[:])
            i2A = cpool.tile([P, sched.totA_cols], dt.int16)
            nc.sync.dma_start(i2A[:], idx2A[:])
            i2B = cpool.tile([P, sched.totB_cols], dt.int16)
            nc.sync.dma_start(i2B[:], idx2B[:])
            adst_all = cpool.tile([P, NT, 8], dt.bfloat16)
            azdst_all = cpool.tile([P, NT, 1], dt.bfloat16)
            o2_all = cpool.tile([P, NT, 16], dt.float32)   # deferred L2 softmax

            # ---- P1: replicated node phase --------------------------------
            XB = 4  # node tiles per x load / per table write
            with (
                tc.tile_pool(name="p1sb", bufs=3) as p1sb,
                tc.tile_pool(name="p1ps", bufs=3, space="PSUM") as p1ps,
            ):
                DR = mybir.MatmulPerfMode.DoubleRow
                for tq in range(0, NODE_TILES, XB):
                    nb = min(XB, NODE_TILES - tq)
                    xt = p1sb.tile([P, 2, nb * P], dt.float8e4, tag="xt")
                    nc.sync.dma_start(xt[:], xT[:, :, tq * P: tq * P + nb * P])
                    rowt = p1sb.tile([P, XB, ROW1], dt.bfloat16, tag="rowt")
                    for u in range(nb):
                        ph = p1ps.tile([P, 512], dt.float32, tag="ph")
                        pa = p1ps.tile([P, 16], dt.float32, tag="pa")
                        lhs = xt[:, :, u * P:(u + 1) * P]
                        nc.tensor.matmul(ph[:], lhs, W1s[:, :, 0:512],
                                         perf_mode=DR, start=True, stop=True)
                        nc.tensor.matmul(pa[:], lhs, W1s[:, :, 512:528],
                                         perf_mode=DR, start=True, stop=True)
                        if u % 2 == 0:
                            nc.scalar.activation(rowt[:, u, 0:512], ph[:],
                                                 AF.Copy, scale=0.0625)
                        else:
                            nc.vector.tensor_scalar(
                                out=rowt[:, u, 0:512], in0=ph[:], scalar1=0.0625,
                                scalar2=None, op0=ALU.mult,
                            )
                        nc.vector.tensor_scalar(
                            out=rowt[:, u, 512:528], in0=pa[:], scalar1=0.0625,
                            scalar2=None, op0=ALU.mult,
                        )
                    # one batched table write per XB tiles (4x fewer sync
                    # queue DMA triggers)
                    nc.sync.dma_start(
                        table1[tq * P:(tq + nb) * P, :].rearrange(
                            "(u p) r -> p u r", p=P),
                        rowt[:, 0:nb, :],
                    )

            # ---- P1.5: per-core a_dst table (SBUF, tile-major) ------------
            if phase not in ("p1",):
                rbase = nc.sync.partition_id() * NPC
                nc.sync.dma_start(
                    adst_own[:, :],
                    table1[bass.ds(rbase, NPCP), 520:528],
                )
                nc.sync.dma_start(
                    adst_all[:],
                    adst_own[:].rearrange("(t p) c -> p t c", p=P),
                )

            # ---- edge phase helper ----------------------------------------
            def edge_phase(layer):
                """layer 1: table1 gathers, 8 heads; layer 2: z-table, 1 head."""
                sub = os.environ.get("GAT_L1SUB", "full")
                if layer == 1:
                    g_src_tab_A, g_src_tab_B = table1, table1[sched.split1:, :]
                    g_elem, g_row = ROW1, ROW1
                    iA, iB = i1A, i1B
                    NH = 8
                    adst_t = adst_all
                else:
                    g_src_tab_A, g_src_tab_B = cc_out, cc_out[sched.zsplit:, :]
                    g_elem, g_row = ROWZ, ROWZ
                    iA, iB = i2A, i2B
                    NH = 1
                    adst_t = azdst_all
                MAXTB = sched.maxtb
                edeep = 3 if layer == 1 else 4   # keep the gather queue fed
                with (
                    tc.tile_pool(name=f"ed{layer}", bufs=edeep) as ep,
                    tc.tile_pool(name=f"dl{layer}", bufs=2 if layer == 1 else 4) as dp,
                    tc.tile_pool(name=f"sl{layer}", bufs=2 if layer == 1 else 3) as sp,
                    tc.tile_pool(name=f"ms{layer}", bufs=2 if layer == 1 else 3) as mp,
                    tc.tile_pool(name=f"eb{layer}", bufs=3 if layer == 1 else 4) as bp,
                    tc.tile_pool(name=f"os{layer}", bufs=8) as op_,
                    tc.tile_pool(name=f"ep{layer}", bufs=2, space="PSUM") as pp,
                    tc.tile_pool(name=f"eo{layer}", bufs=2, space="PSUM") as po,
                    tc.tile_pool(name=f"eq{layer}", bufs=2, space="PSUM") as pq,
                    tc.tile_pool(name=f"ez{layer}", bufs=1, space="PSUM") as pz1,
                ):
                    pending = []
                    for g, tl in enumerate(sched.groups):
                        GB = sched.g_tblk[g]
                        GA = sched.g_ablk[g]
                        nA, nB_ = GA * P, (GB - GA) * P
                        gb = sched.g_base[g]
                        hg = ep.tile([P, GB, g_elem], dt.bfloat16, tag="hg")
                        if nA:
                            nc.gpsimd.dma_gather(
                                hg[:, 0:GA, :], g_src_tab_A[:],
                                iA[:, sched.gA_coloff[g]: sched.gA_coloff[g] + nA // 16],
                                nA, nA, g_elem, single_packet=False,
                                queue_num=(2 * g) % 4,
                            )
                        if nB_:
                            nc.gpsimd.dma_gather(
                                hg[:, GA:GB, :], g_src_tab_B,
                                iB[:, sched.gB_coloff[g]: sched.gB_coloff[g] + nB_ // 16],
                                nB_, nB_, g_elem, elem_step=g_row,
                                single_packet=False,
                                queue_num=(2 * g + 1) % 4,
                            )
                        # transposed dstloc rows for this group (HWDGE stream)
                        dlT = dp.tile([P, GB, P], dt.bfloat16, tag="dlT")
                        nc.sync.dma_start(dlT[:], dstlocTr[:, gb:gb + GB, :])
                        if sub == "gather":
                            continue
                        # per-tile span lists: (in-group block offset, count)
                        tspans = []
                        for ti in tl:
                            ab, bb = int(sched.Ablk[ti]), int(sched.Bblk[ti])
                            if ab + bb == 0:
                                continue
                            spans = []
                            if ab:
                                spans.append((sched.tile_apos[ti], ab))
                            if bb:
                                spans.append((sched.tile_bpos[ti], bb))
                            tspans.append((ti, spans, ab + bb))
                        # ---- stage 1: selection matrices for whole group ----
                        selG = sp.tile([P, GB, P], dt.bfloat16, tag="selS")
                        sel2G = sp.tile([P, GB, P], dt.bfloat16, tag="sel2S")
                        nc.vector.tensor_tensor(
                            out=selG[:],
                            in0=iota_bf[:].rearrange(
                                "p (a b) -> p a b", a=1).to_broadcast([P, GB, P]),
                            in1=dstloc_s[:, gb:gb + GB].rearrange(
                                "p (a b) -> p a b", b=1).to_broadcast([P, GB, P]),
                            op=ALU.is_equal,
                        )
                        nc.vector.tensor_scalar(
                            out=sel2G[:], in0=dlT[:], scalar1=iota_cf[:, 0:1],
                            scalar2=None, op0=ALU.is_equal,
                        )
                        # ---- stage 2: a_dst gathers on PE ----
                        aDsG = pq.tile([P, GB, NH], dt.float32, tag="aDs")
                        for ti, spans, tb in tspans:
                            for (o, n) in spans:
                                for k in range(n):
                                    nc.tensor.matmul(
                                        aDsG[:, o + k, :], sel2G[:, o + k, :],
                                        adst_t[:, ti, :], start=True, stop=True,
                                    )
                        # ---- stage 3: scores for whole group ----
                        if layer == 1:
                            wG = bp.tile([P, GB * 8], dt.bfloat16, tag="wbf")
                        else:
                            wG = bp.tile([P, GB * 1], dt.float32, tag="wt")
                        stG = bp.tile([P, GB * NH], dt.float32, tag="st")
                        lkG = bp.tile([P, GB * NH], dt.float32, tag="lk")
                        a_s = hg[:, :, 512:520] if layer == 1 else hg[:, :, 17:18]
                        nc.vector.tensor_tensor(
                            out=stG[:].rearrange("p (n k) -> p n k", k=NH),
                            in0=a_s, in1=aDsG[:], op=ALU.add,
                        )
                        nc.vector.scalar_tensor_tensor(
                            lkG[:], stG[:], 0.2, stG[:], ALU.mult, ALU.max
                        )
                        nc.scalar.activation(wG[:], lkG[:], AF.Exp)
                        # ---- flush deferred closes of the previous group ----
                        if sub != "blocks":
                            for args in pending:
                                close_tile(*args)
                            pending.clear()
                        if sub == "score":
                            continue
                        # ---- stage 4: Ms builds for whole group ----
                        MsG = mp.tile(
                            [P, GB, 512 if layer == 1 else 17],
                            dt.bfloat16, tag="Ms",
                        )
                        if layer == 1:
                            # h cols interleaved (c*8+h): w broadcast is
                            # middle-dim -> last dim stays packed -> DVE 2x.
                            wbc = wG[:].rearrange(
                                "p (n a b) -> p n a b", n=GB, a=1
                            ).to_broadcast([P, GB, 64, 8])
                            nc.vector.tensor_tensor(
                                out=MsG[:].rearrange("p n (a b) -> p n a b", b=8),
                                in0=hg[:, :, 0:512].rearrange(
                                    "p n (a b) -> p n a b", b=8),
                                in1=wbc,
                                op=ALU.mult,
                            )
                        else:
                            w2bc = wG[:].rearrange(
                                "p (n b) -> p n b", b=1).to_broadcast([P, GB, 17])
                            nc.vector.tensor_tensor(
                                out=MsG[:], in0=hg[:, :, 0:17], in1=w2bc,
                                op=ALU.mult,
                            )
                        # ---- stage 5: aggregation matmuls + psum flush ----
                        for ti, spans, tb in tspans:
                            if layer == 1:
                                psum_o = pp.tile([P, 512], dt.float32, tag="psO")
                                psum_d = po.tile([P, 8], dt.float32, tag="psD")
                            else:
                                psum_o = pp.tile([P, 17], dt.float32, tag="psO")
                                psum_d = None
                            j = 0
                            for (o, n) in spans:
                                for k in range(n):
                                    jj = o + k
                                    first, last = (j == 0), (j == tb - 1)
                                    nc.tensor.matmul(psum_o[:], selG[:, jj, :],
                                                     MsG[:, jj, :],
                                                     start=first, stop=last)
                                    if layer == 1:
                                        nc.tensor.matmul(
                                            psum_d[:], selG[:, jj, :],
                                            wG[:, jj * 8:jj * 8 + 8],
                                            start=first, stop=last,
                                        )
                                    j += 1
                            # flush psums to SBUF right away (scalar engine):
                            # frees the banks and closes read fast SBUF tiles
                            if layer == 1:
                                oS = op_.tile([P, 512], dt.bfloat16, tag="oS")
                                nc.scalar.copy(oS[:], psum_o[:])
                                dS = op_.tile([P, 8], dt.float32, tag="dS")
                                nc.scalar.copy(dS[:], psum_d[:])
                            else:
                                oS = op_.tile([P, 17], dt.float32, tag="oS")
                                nc.scalar.copy(oS[:], psum_o[:])
                                dS = None
                            pending.append((layer, ti, oS, dS, bp, pz1, pz1))
                    # drain the last group's closes
                    if sub not in ("gather", "score", "blocks"):
                        for args in pending:
                            close_tile(*args)
                        pending.clear()

            # ---- tile close -------------------------------------------------
            def close_tile(layer, ti, oS, dS, bp, ptp, pzp):
                if layer == 1:
                    r = bp.tile([P, 8], dt.float32, tag="r")
                    nc.vector.reciprocal(r[:], dS[:])
                    o1 = bp.tile([P, 512], dt.bfloat16, tag="o1")
                    o13 = o1[:].rearrange("p (c h) -> p c h", h=8)
                    rbc = r[:].rearrange("p (a h) -> p a h", a=1).to_broadcast([P, 64, 8])
                    nc.vector.tensor_tensor(
                        out=o13,
                        in0=oS[:].rearrange("p (c h) -> p c h", h=8),
                        in1=rbc, op=ALU.mult,
                    )
                    # elu: h2 = max(o1,0) + exp(min(o1,0)) - 1
                    u = bp.tile([P, 512], dt.bfloat16, tag="u")
                    nc.vector.tensor_scalar_min(u[:], o1[:], 0.0)
                    e1 = bp.tile([P, 512], dt.bfloat16, tag="e1")
                    nc.scalar.activation(e1[:], u[:], AF.Exp)
                    rv = bp.tile([P, 512], dt.bfloat16, tag="rv")
                    nc.vector.tensor_scalar_max(rv[:], o1[:], 0.0)
                    h2 = bp.tile([P, 512], dt.bfloat16, tag="h2")
                    nc.vector.scalar_tensor_tensor(
                        h2[:], e1[:], -1.0, rv[:], ALU.add, ALU.add
                    )
                    # transpose h2 -> z matmuls
                    pz = pzp.tile([P, 18], dt.float32, tag="psZ")
                    for c in range(4):
                        ptr = ptp.tile([P, P], dt.bfloat16, tag="psT")
                        nc.tensor.transpose(ptr[:], h2[:, c * P:(c + 1) * P], ident[:])
                        h2T = bp.tile([P, P], dt.bfloat16, tag="h2T")
                        nc.scalar.copy(h2T[:], ptr[:])
                        nc.tensor.matmul(pz[:], h2T[:], wz[:, c, :], start=(c == 0), stop=(c == 3))
                    # z-row layout: [z 16 | 1.0 | a_src2 | a_dst2 | junk]
                    zrow = bp.tile([P, ROWZ], dt.bfloat16, tag="zrow")
                    nc.vector.tensor_copy(zrow[:, 0:16], pz[:, 0:16])
                    nc.vector.memset(zrow[:, 16:17], 1.0)
                    nc.vector.tensor_copy(zrow[:, 17:19], pz[:, 16:18])
                    nc.sync.dma_start(cc_in[ti * P:(ti + 1) * P, :], zrow[:])
                else:
                    # defer log_softmax to one batched pass (avoids Exp<->Ln
                    # activation-table thrash against the edge-phase Exp)
                    r2 = bp.tile([P, 1], dt.float32, tag="r2")
                    nc.vector.reciprocal(r2[:], oS[:, 16:17])
                    nc.vector.tensor_scalar_mul(
                        o2_all[:, ti, :], oS[:, 0:16], r2[:, 0:1])

            if phase not in ("p1", "p15"):
                edge_phase(1)

            if phase in ("cc", "full"):
                # ---- z-table exchange -------------------------------------
                if n_cores == 1:
                    nc.sync.dma_start(cc_out[:, :], cc_in[:, :])
                else:
                    nc.gpsimd.collective_compute(
                        "AllGather", ALU.bypass,
                        ins=[cc_in[:]], outs=[cc_out[:]],
                        replica_groups=[list(range(n_cores))],
                    )

            if phase == "full":
                nc.sync.dma_start(
                    azdst_all[:],
                    cc_in[:, 18:19].rearrange("(t p) c -> p t c", p=P),
                )
                edge_phase(2)
                # batched log_softmax over all dst tiles (2 act-table loads)
                with tc.tile_pool(name="fin", bufs=1) as fp:
                    mx = fp.tile([P, NT, 1], dt.float32)
                    nc.vector.tensor_reduce(
                        mx[:], o2_all[:], axis=mybir.AxisListType.X, op=ALU.max)
                    o2m = fp.tile([P, NT, 16], dt.float32)
                    nc.vector.tensor_tensor(
                        out=o2m[:], in0=o2_all[:],
                        in1=mx[:].to_broadcast([P, NT, 16]), op=ALU.subtract)
                    ex = fp.tile([P, NT, 16], dt.float32)
                    nc.scalar.activation(ex[:], o2m[:], AF.Exp)
                    ssum = fp.tile([P, NT, 1], dt.float32)
                    nc.vector.tensor_reduce(
                        ssum[:], ex[:], axis=mybir.AxisListType.X, op=ALU.add)
                    lse = fp.tile([P, NT, 1], dt.float32)
                    nc.scalar.activation(lse[:], ssum[:], AF.Ln)
                    res = fp.tile([P, NT, 16], dt.float32)
                    nc.vector.tensor_tensor(
                        out=res[:], in0=o2m[:],
                        in1=lse[:].to_broadcast([P, NT, 16]), op=ALU.subtract)
                    nc.sync.dma_start(
                        out_shard[:].rearrange("(t p) c -> p t c", p=P), res[:])

    nc.compile()
    return nc


# ----------------------------------------------------------------------------
# host entry
# ----------------------------------------------------------------------------

def _blockdiag(att, heads, hid):
    """[heads, hid] -> [heads*hid, heads] block diagonal."""
    out = np.zeros((heads * hid, max(heads, 1)), np.float32)
    for h in range(heads):
        out[h * hid:(h + 1) * hid, h] = att[h]
    return out


def prepare_inputs(inputs, sched: Schedule):
    x = np.asarray(inputs["x"], np.float32)
    ei = np.asarray(inputs["edge_index"])
    W1 = np.asarray(inputs["W1"], np.float32)
    as1 = np.asarray(inputs["att_src1"], np.float32)
    ad1 = np.asarray(inputs["att_dst1"], np.float32)
    W2 = np.asarray(inputs["W2"], np.float32)
    as2 = np.asarray(inputs["att_src2"], np.float32)
    ad2 = np.asarray(inputs["att_dst2"], np.float32)

    N, IN = x.shape
    TR = sched.table_rows
    xp = np.zeros((TR, IN), np.float32)
    xp[:N] = x
    F8 = ml_dtypes.float8_e4m3
    xTb = np.ascontiguousarray(
        xp.T.reshape(2, P, TR).transpose(1, 0, 2)).astype(F8)
    # interleave h columns: new col j = c*8 + h  <->  old col h*64 + c
    perm = np.array([(j % 8) * 64 + (j // 8) for j in range(512)], np.int64)
    acat = np.concatenate(
        [_blockdiag(as1, 8, 64), _blockdiag(ad1, 8, 64)], axis=1)  # [512, 16]
    wcat = W1 @ acat                                     # [256, 16]
    W1c = np.concatenate([W1[:, perm], wcat], axis=1)    # [256, 528]
    W1b = np.ascontiguousarray(
        (W1c * 16.0).reshape(2, P, 528).transpose(1, 0, 2)).astype(F8)
    att2b = np.concatenate([as2.T, ad2.T], axis=1)       # [16, 2]
    wzf = np.concatenate([W2[perm, :], W2[perm, :] @ att2b], axis=1)  # [512, 18]
    wzb = np.ascontiguousarray(wzf.reshape(4, P, 18).transpose(1, 0, 2)).astype(BF)

    shared = dict(xT=xTb, W1r=W1b, wzr=wzb)
    maps = []
    for c in range(sched.n_cores):
        pc = sched.per_core[c]
        m = dict(shared)
        m.update(
            idx1A=pc["idx1A"], idx1B=pc["idx1B"], idx2A=pc["idx2A"],
            idx2B=pc["idx2B"], dstloc=pc["dstloc"], dstlocT=pc["dstlocT"],
        )
        maps.append(m)
    return maps


_LAST_RESULT = {}


def kernel(**inputs):
    from concourse.bass_utils import run_bass_kernel_spmd

    x = np.asarray(inputs["x"], np.float32)
    ei = np.asarray(inputs["edge_index"], np.int64)
    N = x.shape[0]
    n_cores = 8
    loops = np.arange(N, dtype=np.int64)
    src = np.concatenate([ei[0], loops])
    dst = np.concatenate([ei[1], loops])

    sched = Schedule(src, dst, N, n_cores)
    phase = os.environ.get("GAT_PHASE", "full")
    nc = build_program(sched, n_cores, phase=phase)
    in_maps = prepare_inputs(inputs, sched)

    trace = bool(int(os.environ.get("GAT_TRACE", "0")))
    res = run_bass_kernel_spmd(
        nc, in_maps, core_ids=list(range(n_cores)), trace=trace,
    )
    _LAST_RESULT["res"] = res

    out = np.zeros((N, 16), np.float32)
    for c in range(n_cores):
        sh = res.results[c]["out_shard"]
        n0 = c * sched.npc
        out[n0:n0 + sched.npc] = sh[: sched.npc]
    return out



# revision 31
# speedup vs baseline: 1.0841x; 1.0228x over previous
"""2-layer GAT (GATConv x2 + log_softmax) on 8 Trainium2 NeuronCores.

Strategy (dst-sharded message passing):
  - Nodes are sharded contiguously across 8 cores (6250 each); every edge is
    owned by the core owning its dst node.  Edges are grouped by dst tile
    (128 dst nodes), split into A/B halves by src id (so gather indices fit
    int16), padded to 128-edge blocks with a cross-core-uniform schedule so
    all 8 cores run one SPMD program.
  - Layer-1 node phase is replicated: every core computes h = x@W1 (bf16,
    fp32 accum) for ALL nodes and writes a gather table
    [h(512) | a_src(8) | a_dst(8) | pad] bf16 per node.
  - Edge phase per 128-edge block: dma_gather rows by src, build a 0/1
    selection matrix SelT[e,d] = (dst_local[e] == d) on DVE, per-head
    weight multiply, then PE matmul SelT.T @ M accumulates the segment sum
    (and the softmax denominator) in PSUM per dst tile.
  - Scores: exp(leaky_relu(a_src[src] + a_dst[dst])) with a_dst gathered
    from a per-core table; softmax normalization is applied per dst tile
    after aggregation (alpha = w/denom pulled out of the edge sum).
  - Layer 2 (1 head, 16 ch) reuses the same block structure; the small
    z-table is exchanged with an AllGather collective.
"""
import os
import math
import numpy as np
import ml_dtypes

import concourse.bass as bass
import concourse.mybir as mybir
import concourse.tile as tile
import concourse.bacc as bacc
from concourse.masks import make_identity
from concourse.library_config import mlp

BF = ml_dtypes.bfloat16
dt = mybir.dt
AF = mybir.ActivationFunctionType
ALU = mybir.AluOpType

P = 128
ROW1 = 640     # table1 cols (bf16): [h 512 | a_src 8 | a_dst 8 | pad]
ROWZ = 128     # z-table cols (bf16): [z 16 | a_src2 1 | a_dst2 1 | pad]
BLKCAP = 22    # max blocks per gather group


# ----------------------------------------------------------------------------
# host-side schedule construction
# ----------------------------------------------------------------------------

def _wrap_idx(vals, slots):
    """Pad `vals` with 0 to `slots`, wrap into [128, slots/16] int16 layout."""
    v = np.zeros(slots, np.int64)
    v[: len(vals)] = vals
    a = v.reshape(-1, 16).T  # [16, slots/16]
    return np.tile(a, (8, 1)).astype(np.int16)


class Schedule:
    """Cross-core-uniform block schedule + per-core index arrays."""

    def __init__(self, src, dst, n_nodes, n_cores, force_split=None):
        self.n_nodes = n_nodes
        self.n_cores = n_cores
        self.npc = n_nodes // n_cores                 # real nodes per core
        self.nt = (self.npc + P - 1) // P             # dst tiles per core
        self.npcp = self.nt * P                       # padded nodes per core
        self.ntot_p = ((n_nodes + P - 1) // P) * P if n_cores == 1 else None
        # padded global table rows (node-id indexed)
        self.table_rows = ((n_nodes + P - 1) // P) * P
        self.table_rows = max(self.table_rows, self.npcp * n_cores)
        self.zrows = self.npcp * n_cores              # z-table rows (zid indexed)

        # split for int16 gathers: src <= SPLIT1-1 -> table A half;
        # zid(src) <= 32767 must also hold.
        if force_split is not None:
            self.split1 = force_split
        elif self.table_rows <= 32768 and self.zrows <= 32768:
            self.split1 = self.table_rows  # no B half
        else:
            # largest s with s-1 <= 32767 and zid(s-1) <= 32767
            s = min(32768, self.n_nodes)
            while s > 0:
                n = s - 1
                zid = (n // self.npc) * self.npcp + (n % self.npc)
                if zid <= 32767:
                    break
                s -= 1
            self.split1 = s
        self.zsplit = ((self.split1 - 1) // self.npc) * self.npcp + (
            (self.split1 - 1) % self.npc
        ) + 1 if self.split1 < self.table_rows else self.zrows

        core = dst // self.npc
        loc = dst - core * self.npc
        t = loc // P
        dloc = loc % P
        isB = src >= self.split1

        nc_, nt_ = n_cores, self.nt
        # counts[core, tile, {A,B}]
        key = (core * nt_ + t) * 2 + isB
        cnt = np.bincount(key, minlength=nc_ * nt_ * 2).reshape(nc_, nt_, 2)
        mx = cnt.max(axis=0)                            # [nt, 2]
        self.Ablk = np.ceil(mx[:, 0] / P).astype(int)
        self.Bblk = np.ceil(mx[:, 1] / P).astype(int)
        self.TBlk = self.Ablk + self.Bblk

        # groups: consecutive tiles, sum(TBlk) <= BLKCAP
        self.groups = []
        cur, acc = [], 0
        for ti in range(nt_):
            tb = int(self.TBlk[ti])
            if cur and acc + tb > BLKCAP:
                self.groups.append(cur)
                cur, acc = [], 0
            cur.append(ti)
            acc += tb
        if cur:
            self.groups.append(cur)

        # canonical block order & per-tile positions within group buffers
        # group buffer layout: [A-blocks of each tile in order, then B-blocks]
        self.g_ablk = []   # per group: total A blocks
        self.g_tblk = []   # per group: total blocks
        self.tile_apos = {}  # tile -> in-group A block offset
        self.tile_bpos = {}  # tile -> in-group block offset of its B blocks
        self.g_base = []     # per group: global block offset
        nblocks = 0
        for g, tl in enumerate(self.groups):
            ga = int(sum(self.Ablk[ti] for ti in tl))
            gt = int(sum(self.TBlk[ti] for ti in tl))
            self.g_ablk.append(ga)
            self.g_tblk.append(gt)
            ao = 0
            bo = ga
            for ti in tl:
                self.tile_apos[ti] = ao
                self.tile_bpos[ti] = bo
                ao += int(self.Ablk[ti])
                bo += int(self.Bblk[ti])
            self.g_base.append(nblocks)
            nblocks += gt
        self.nblocks = nblocks

        self.maxtb = int(self.TBlk.max())

        # per-core arrays
        # order edges by (core, tile, isB) stably
        order = np.lexsort((isB, t, core))
        self.per_core = []
        for c in range(nc_):
            m0 = order[core[order] == c]
            idx1A_cols, idx1B_cols, idx2A_cols, idx2B_cols = [], [], [], []
            dstloc = np.full((P, nblocks), 999.0, np.float32)
            for g, tl in enumerate(self.groups):
                a_src_l, b_src_l = [], []
                dl_A, dl_B = [], []
                for ti in tl:
                    e = m0[t[m0] == ti]
                    eA = e[~isB[e]]
                    eB = e[isB[e]]
                    nA = int(self.Ablk[ti]) * P
                    nB = int(self.Bblk[ti]) * P
                    sA = np.zeros(nA, np.int64)
                    sA[: len(eA)] = src[eA]
                    sB = np.zeros(nB, np.int64)
                    sB[: len(eB)] = src[eB] - self.split1
                    lA = np.full(nA, 999.0, np.float32)
                    lA[: len(eA)] = dloc[eA]
                    lB = np.full(nB, 999.0, np.float32)
                    lB[: len(eB)] = dloc[eB]
                    a_src_l.append(sA)
                    b_src_l.append(sB)
                    dl_A.append(lA)
                    dl_B.append(lB)
                gsA = np.concatenate(a_src_l) if a_src_l else np.zeros(0, np.int64)
                gsB = np.concatenate(b_src_l) if b_src_l else np.zeros(0, np.int64)
                gdl = np.concatenate(dl_A + dl_B) if (dl_A or dl_B) else np.zeros(0, np.float32)
                # L2 indices: zid mapping of global src
                def zid_of(v):
                    vv = np.asarray(v, np.int64)
                    return (vv // self.npc) * self.npcp + (vv % self.npc)
                g2A = zid_of(gsA)                       # gsA holds global src (pads=0)
                g2B = zid_of(gsB + self.split1) - self.zsplit
                idx1A_cols.append(_wrap_idx(gsA, len(gsA)))
                idx1B_cols.append(_wrap_idx(gsB, len(gsB)))
                idx2A_cols.append(_wrap_idx(g2A, len(g2A)))
                idx2B_cols.append(_wrap_idx(g2B, len(g2B)))
                gb = self.g_base[g]
                dstloc[:, gb : gb + self.g_tblk[g]] = gdl.reshape(-1, P).T
            cat = lambda ls: (
                np.concatenate(ls, axis=1) if ls and sum(x.shape[1] for x in ls) else np.zeros((P, 1), np.int16)
            )
            # sel2 one-hot matrix, shipped pre-built (same bytes as the old
            # replicated dstlocT stream, zero DVE on device):
            # sel2[p, blk, e] = (dst_local(edge e of blk) == p)
            dl = dstloc.T.copy()                       # [nblocks, P(edge)]
            dl[dl == 999.0] = 512.0
            dstlocT = (dl[None, :, :] ==
                       np.arange(P, dtype=np.float32)[:, None, None]).astype(BF)
            self.per_core.append(
                dict(
                    idx1A=cat(idx1A_cols), idx1B=cat(idx1B_cols),
                    idx2A=cat(idx2A_cols), idx2B=cat(idx2B_cols),
                    dstloc=dstloc, dstlocT=dstlocT,
                )
            )
        # column offsets per group in the concatenated idx arrays
        self.gA_coloff, self.gB_coloff = [], []
        a = b = 0
        for g in range(len(self.groups)):
            self.gA_coloff.append(a)
            self.gB_coloff.append(b)
            a += (self.g_ablk[g] * P) // 16
            b += ((self.g_tblk[g] - self.g_ablk[g]) * P) // 16
        self.totA_cols = max(a, 1)
        self.totB_cols = max(b, 1)


# ----------------------------------------------------------------------------
# device program
# ----------------------------------------------------------------------------

def build_program(sched: Schedule, n_cores: int, phase: str = 'full'):
    """Build the SPMD Bass/Tile program for the given schedule."""
    nc = bacc.Bacc(None, target_bir_lowering=False, debug=True, num_devices=n_cores,
                   num_swdge_queues=4)

    TR = sched.table_rows
    ZR = sched.zrows
    NT = sched.nt
    NPC, NPCP = sched.npc, sched.npcp
    NODE_TILES = TR // P

    # ---- inputs -------------------------------------------------------------
    xT = nc.dram_tensor("xT", [P, 2, TR], dt.float8e4, kind="ExternalInput")
    # W1 with interleaved out-cols (c*8+h) + fused [Asrc|Adst] cols -> 528
    W1r = nc.dram_tensor("W1r", [P, 2, 528], dt.float8e4, kind="ExternalInput")
    # wz = [W2p | W2p @ [as2|ad2]] with rows in interleaved order: [512, 18]
    wzr = nc.dram_tensor("wzr", [P, 4, 18], dt.bfloat16, kind="ExternalInput")
    idx1A = nc.dram_tensor("idx1A", [P, sched.totA_cols], dt.int16, kind="ExternalInput")
    idx1B = nc.dram_tensor("idx1B", [P, sched.totB_cols], dt.int16, kind="ExternalInput")
    idx2A = nc.dram_tensor("idx2A", [P, sched.totA_cols], dt.int16, kind="ExternalInput")
    idx2B = nc.dram_tensor("idx2B", [P, sched.totB_cols], dt.int16, kind="ExternalInput")
    dstlocr = nc.dram_tensor("dstloc", [P, sched.nblocks], dt.float32, kind="ExternalInput")
    dstlocTr = nc.dram_tensor("dstlocT", [P, sched.nblocks, P], dt.bfloat16, kind="ExternalInput")
    out_shard = nc.dram_tensor("out_shard", [NPCP, 16], dt.float32, kind="ExternalOutput")

    with tile.TileContext(nc) as tc:
        nc.gpsimd.load_library(mlp)
        with (
            tc.tile_pool(name="dram", bufs=1, space="DRAM") as dram,
            tc.tile_pool(name="const", bufs=1) as cpool,
        ):
            table1 = dram.tile([TR, ROW1], dt.bfloat16)
            adst_own = dram.tile([NPCP, 8], dt.bfloat16)
            cc_in = dram.tile([NPCP, ROWZ], dt.bfloat16)
            cc_out = dram.tile([ZR, ROWZ], dt.bfloat16,
                               addr_space=("Shared" if n_cores > 1 else "Local"))

            # ---- constants -------------------------------------------------
            iota_i = cpool.tile([P, P], dt.int32)
            nc.gpsimd.iota(iota_i[:], pattern=[[1, P]], base=0, channel_multiplier=0)
            iota_bf = cpool.tile([P, P], dt.bfloat16)
            nc.vector.tensor_copy(iota_bf[:], iota_i[:])
            iota_ci = cpool.tile([P, 1], dt.int32)
            nc.gpsimd.iota(iota_ci[:], pattern=[[0, 1]], base=0, channel_multiplier=1)
            iota_cf = cpool.tile([P, 1], dt.float32)
            nc.vector.tensor_copy(iota_cf[:], iota_ci[:])
            ident = cpool.tile([P, P], dt.bfloat16)
            make_identity(nc, ident[:])

            W1s = cpool.tile([P, 2, 528], dt.float8e4)
            nc.sync.dma_start(W1s[:], W1r[:])
            wz = cpool.tile([P, 4, 18], dt.bfloat16)
            nc.sync.dma_start(wz[:], wzr[:])
            dstloc_s = cpool.tile([P, sched.nblocks], dt.float32)
            nc.sync.dma_start(dstloc_s[:], dstlocr[:])
            i1A = cpool.tile([P, sched.totA_cols], dt.int16)
            nc.sync.dma_start(i1A[:], idx1A[:])
            i1B = cpool.tile([P, sched.totB_cols], dt.int16)
            nc.sync.dma_start(i1B[:], idx1B[:])
            i2A = cpool.tile([P, sched.totA_cols], dt.int16)
            nc.sync.dma_start(i2A[:], idx2A[:])
            i2B = cpool.tile([P, sched.totB_cols], dt.int16)
            nc.sync.dma_start(i2B[:], idx2B[:])
            adst_all = cpool.tile([P, NT, 8], dt.bfloat16)
            azdst_all = cpool.tile([P, NT, 1], dt.bfloat16)
            o2_all = cpool.tile([P, NT, 16], dt.float32)   # deferred L2 softmax

            # ---- P1: replicated node phase --------------------------------
            XB = 4  # node tiles per x load / per table write
            with (
                tc.tile_pool(name="p1sb", bufs=3) as p1sb,
                tc.tile_pool(name="p1ps", bufs=3, space="PSUM") as p1ps,
            ):
                DR = mybir.MatmulPerfMode.DoubleRow
                for tq in range(0, NODE_TILES, XB):
                    nb = min(XB, NODE_TILES - tq)
                    xt = p1sb.tile([P, 2, nb * P], dt.float8e4, tag="xt")
                    nc.sync.dma_start(xt[:], xT[:, :, tq * P: tq * P + nb * P])
                    rowt = p1sb.tile([P, XB, ROW1], dt.bfloat16, tag="rowt")
                    for u in range(nb):
                        ph = p1ps.tile([P, 512], dt.float32, tag="ph")
                        pa = p1ps.tile([P, 16], dt.float32, tag="pa")
                        lhs = xt[:, :, u * P:(u + 1) * P]
                        nc.tensor.matmul(ph[:], lhs, W1s[:, :, 0:512],
                                         perf_mode=DR, start=True, stop=True)
                        nc.tensor.matmul(pa[:], lhs, W1s[:, :, 512:528],
                                         perf_mode=DR, start=True, stop=True)
                        if u % 2 == 0:
                            nc.scalar.activation(rowt[:, u, 0:512], ph[:],
                                                 AF.Copy, scale=0.0625)
                        else:
                            nc.vector.tensor_scalar(
                                out=rowt[:, u, 0:512], in0=ph[:], scalar1=0.0625,
                                scalar2=None, op0=ALU.mult,
                            )
                        nc.vector.tensor_scalar(
                            out=rowt[:, u, 512:528], in0=pa[:], scalar1=0.0625,
                            scalar2=None, op0=ALU.mult,
                        )
                    # one batched table write per XB tiles (4x fewer sync
                    # queue DMA triggers)
                    nc.sync.dma_start(
                        table1[tq * P:(tq + nb) * P, :].rearrange(
                            "(u p) r -> p u r", p=P),
                        rowt[:, 0:nb, :],
                    )

            # ---- P1.5: per-core a_dst table (SBUF, tile-major) ------------
            if phase not in ("p1",):
                rbase = nc.sync.partition_id() * NPC
                nc.sync.dma_start(
                    adst_own[:, :],
                    table1[bass.ds(rbase, NPCP), 520:528],
                )
                nc.sync.dma_start(
                    adst_all[:],
                    adst_own[:].rearrange("(t p) c -> p t c", p=P),
                )

            # ---- edge phase helper ----------------------------------------
            def edge_phase(layer):
                """layer 1: table1 gathers, 8 heads; layer 2: z-table, 1 head."""
                sub = os.environ.get("GAT_L1SUB", "full")
                if layer == 1:
                    g_src_tab_A, g_src_tab_B = table1, table1[sched.split1:, :]
                    g_elem, g_row = ROW1, ROW1
                    iA, iB = i1A, i1B
                    NH = 8
                    adst_t = adst_all
                else:
                    g_src_tab_A, g_src_tab_B = cc_out, cc_out[sched.zsplit:, :]
                    g_elem, g_row = ROWZ, ROWZ
                    iA, iB = i2A, i2B
                    NH = 1
                    adst_t = azdst_all
                MAXTB = sched.maxtb
                edeep = 3 if layer == 1 else 4   # keep the gather queue fed
                with (
                    tc.tile_pool(name=f"ed{layer}", bufs=edeep) as ep,
                    tc.tile_pool(name=f"dl{layer}", bufs=2 if layer == 1 else 4) as dp,
                    tc.tile_pool(name=f"sl{layer}", bufs=2 if layer == 1 else 3) as sp,
                    tc.tile_pool(name=f"ms{layer}", bufs=2 if layer == 1 else 3) as mp,
                    tc.tile_pool(name=f"eb{layer}", bufs=3 if layer == 1 else 4) as bp,
                    tc.tile_pool(name=f"os{layer}", bufs=8) as op_,
                    tc.tile_pool(name=f"ep{layer}", bufs=2, space="PSUM") as pp,
                    tc.tile_pool(name=f"eo{layer}", bufs=2, space="PSUM") as po,
                    tc.tile_pool(name=f"eq{layer}", bufs=2, space="PSUM") as pq,
                    tc.tile_pool(name=f"ez{layer}", bufs=1, space="PSUM") as pz1,
                ):
                    pending = []
                    for g, tl in enumerate(sched.groups):
                        GB = sched.g_tblk[g]
                        GA = sched.g_ablk[g]
                        nA, nB_ = GA * P, (GB - GA) * P
                        gb = sched.g_base[g]
                        hg = ep.tile([P, GB, g_elem], dt.bfloat16, tag="hg")
                        if nA:
                            nc.gpsimd.dma_gather(
                                hg[:, 0:GA, :], g_src_tab_A[:],
                                iA[:, sched.gA_coloff[g]: sched.gA_coloff[g] + nA // 16],
                                nA, nA, g_elem, single_packet=False,
                                queue_num=(2 * g) % 4,
                            )
                        if nB_:
                            nc.gpsimd.dma_gather(
                                hg[:, GA:GB, :], g_src_tab_B,
                                iB[:, sched.gB_coloff[g]: sched.gB_coloff[g] + nB_ // 16],
                                nB_, nB_, g_elem, elem_step=g_row,
                                single_packet=False,
                                queue_num=(2 * g + 1) % 4,
                            )
                        # transposed dstloc rows for this group (HWDGE stream)
                        dlT = dp.tile([P, GB, P], dt.bfloat16, tag="dlT")
                        nc.sync.dma_start(dlT[:], dstlocTr[:, gb:gb + GB, :])
                        if sub == "gather":
                            continue
                        # per-tile span lists: (in-group block offset, count)
                        tspans = []
                        for ti in tl:
                            ab, bb = int(sched.Ablk[ti]), int(sched.Bblk[ti])
                            if ab + bb == 0:
                                continue
                            spans = []
                            if ab:
                                spans.append((sched.tile_apos[ti], ab))
                            if bb:
                                spans.append((sched.tile_bpos[ti], bb))
                            tspans.append((ti, spans, ab + bb))
                        # ---- stage 1: selection matrices for whole group ----
                        selG = sp.tile([P, GB, P], dt.bfloat16, tag="selS")
                        nc.vector.tensor_tensor(
                            out=selG[:],
                            in0=iota_bf[:].rearrange(
                                "p (a b) -> p a b", a=1).to_broadcast([P, GB, P]),
                            in1=dstloc_s[:, gb:gb + GB].rearrange(
                                "p (a b) -> p a b", b=1).to_broadcast([P, GB, P]),
                            op=ALU.is_equal,
                        )
                        sel2G = dlT
                        # ---- stage 2: a_dst gathers on PE ----
                        aDsG = pq.tile([P, GB, NH], dt.float32, tag="aDs")
                        for ti, spans, tb in tspans:
                            for (o, n) in spans:
                                for k in range(n):
                                    nc.tensor.matmul(
                                        aDsG[:, o + k, :], sel2G[:, o + k, :],
                                        adst_t[:, ti, :], start=True, stop=True,
                                    )
                        # ---- stage 3: scores for whole group ----
                        if layer == 1:
                            wG = bp.tile([P, GB * 8], dt.bfloat16, tag="wbf")
                        else:
                            wG = bp.tile([P, GB * 1], dt.float32, tag="wt")
                        stG = bp.tile([P, GB * NH], dt.float32, tag="st")
                        lkG = bp.tile([P, GB * NH], dt.float32, tag="lk")
                        a_s = hg[:, :, 512:520] if layer == 1 else hg[:, :, 17:18]
                        nc.vector.tensor_tensor(
                            out=stG[:].rearrange("p (n k) -> p n k", k=NH),
                            in0=a_s, in1=aDsG[:], op=ALU.add,
                        )
                        nc.vector.scalar_tensor_tensor(
                            lkG[:], stG[:], 0.2, stG[:], ALU.mult, ALU.max
                        )
                        nc.scalar.activation(wG[:], lkG[:], AF.Exp)
                        # ---- flush deferred closes of the previous group ----
                        if sub != "blocks":
                            for args in pending:
                                close_tile(*args)
                            pending.clear()
                        if sub == "score":
                            continue
                        # ---- stage 4: Ms builds for whole group ----
                        MsG = mp.tile(
                            [P, GB, 512 if layer == 1 else 17],
                            dt.bfloat16, tag="Ms",
                        )
                        if layer == 1:
                            # h cols interleaved (c*8+h): w broadcast is
                            # middle-dim -> last dim stays packed -> DVE 2x.
                            wbc = wG[:].rearrange(
                                "p (n a b) -> p n a b", n=GB, a=1
                            ).to_broadcast([P, GB, 64, 8])
                            nc.vector.tensor_tensor(
                                out=MsG[:].rearrange("p n (a b) -> p n a b", b=8),
                                in0=hg[:, :, 0:512].rearrange(
                                    "p n (a b) -> p n a b", b=8),
                                in1=wbc,
                                op=ALU.mult,
                            )
                        else:
                            w2bc = wG[:].rearrange(
                                "p (n b) -> p n b", b=1).to_broadcast([P, GB, 17])
                            nc.vector.tensor_tensor(
                                out=MsG[:], in0=hg[:, :, 0:17], in1=w2bc,
                                op=ALU.mult,
                            )
                        # ---- stage 5: aggregation matmuls + psum flush ----
                        for ti, spans, tb in tspans:
                            if layer == 1:
                                psum_o = pp.tile([P, 512], dt.float32, tag="psO")
                                psum_d = po.tile([P, 8], dt.float32, tag="psD")
                            else:
                                psum_o = pp.tile([P, 17], dt.float32, tag="psO")
                                psum_d = None
                            j = 0
                            for (o, n) in spans:
                                for k in range(n):
                                    jj = o + k
                                    first, last = (j == 0), (j == tb - 1)
                                    nc.tensor.matmul(psum_o[:], selG[:, jj, :],
                                                     MsG[:, jj, :],
                                                     start=first, stop=last)
                                    if layer == 1:
                                        nc.tensor.matmul(
                                            psum_d[:], selG[:, jj, :],
                                            wG[:, jj * 8:jj * 8 + 8],
                                            start=first, stop=last,
                                        )
                                    j += 1
                            # flush psums to SBUF right away (scalar engine):
                            # frees the banks and closes read fast SBUF tiles
                            if layer == 1:
                                oS = op_.tile([P, 512], dt.bfloat16, tag="oS")
                                nc.scalar.copy(oS[:], psum_o[:])
                                dS = op_.tile([P, 8], dt.float32, tag="dS")
                                nc.scalar.copy(dS[:], psum_d[:])
                            else:
                                oS = op_.tile([P, 17], dt.float32, tag="oS")
                                nc.scalar.copy(oS[:], psum_o[:])
                                dS = None
                            pending.append((layer, ti, oS, dS, bp, pz1, pz1))
                    # drain the last group's closes
                    if sub not in ("gather", "score", "blocks"):
                        for args in pending:
                            close_tile(*args)
                        pending.clear()

            # ---- tile close -------------------------------------------------
            def close_tile(layer, ti, oS, dS, bp, ptp, pzp):
                if layer == 1:
                    r = bp.tile([P, 8], dt.bfloat16, tag="r")
                    with nc.allow_low_precision(reason="softmax denom, bf16 ok"):
                        nc.vector.reciprocal(r[:], dS[:])
                    o1 = bp.tile([P, 512], dt.bfloat16, tag="o1")
                    o13 = o1[:].rearrange("p (c h) -> p c h", h=8)
                    rbc = r[:].rearrange("p (a h) -> p a h", a=1).to_broadcast([P, 64, 8])
                    nc.vector.tensor_tensor(
                        out=o13,
                        in0=oS[:].rearrange("p (c h) -> p c h", h=8),
                        in1=rbc, op=ALU.mult,
                    )
                    # elu: h2 = max(o1,0) + exp(min(o1,0)) - 1
                    u = bp.tile([P, 512], dt.bfloat16, tag="u")
                    nc.vector.tensor_scalar_min(u[:], o1[:], 0.0)
                    e1 = bp.tile([P, 512], dt.bfloat16, tag="e1")
                    nc.scalar.activation(e1[:], u[:], AF.Exp)
                    rv = bp.tile([P, 512], dt.bfloat16, tag="rv")
                    nc.vector.tensor_scalar_max(rv[:], o1[:], 0.0)
                    h2 = bp.tile([P, 512], dt.bfloat16, tag="h2")
                    nc.vector.scalar_tensor_tensor(
                        h2[:], e1[:], -1.0, rv[:], ALU.add, ALU.add
                    )
                    # transpose h2 -> z matmuls
                    pz = pzp.tile([P, 18], dt.float32, tag="psZ")
                    for c in range(4):
                        ptr = ptp.tile([P, P], dt.bfloat16, tag="psT")
                        nc.tensor.transpose(ptr[:], h2[:, c * P:(c + 1) * P], ident[:])
                        h2T = bp.tile([P, P], dt.bfloat16, tag="h2T")
                        nc.scalar.copy(h2T[:], ptr[:])
                        nc.tensor.matmul(pz[:], h2T[:], wz[:, c, :], start=(c == 0), stop=(c == 3))
                    # z-row layout: [z 16 | 1.0 | a_src2 | a_dst2 | junk]
                    zrow = bp.tile([P, ROWZ], dt.bfloat16, tag="zrow")
                    nc.vector.tensor_copy(zrow[:, 0:16], pz[:, 0:16])
                    nc.vector.memset(zrow[:, 16:17], 1.0)
                    nc.vector.tensor_copy(zrow[:, 17:19], pz[:, 16:18])
                    nc.sync.dma_start(cc_in[ti * P:(ti + 1) * P, :], zrow[:])
                else:
                    # defer log_softmax to one batched pass (avoids Exp<->Ln
                    # activation-table thrash against the edge-phase Exp)
                    r2 = bp.tile([P, 1], dt.float32, tag="r2")
                    nc.vector.reciprocal(r2[:], oS[:, 16:17])
                    nc.vector.tensor_scalar_mul(
                        o2_all[:, ti, :], oS[:, 0:16], r2[:, 0:1])

            if phase not in ("p1", "p15"):
                edge_phase(1)

            if phase in ("cc", "full"):
                # ---- z-table exchange -------------------------------------
                if n_cores == 1:
                    nc.sync.dma_start(cc_out[:, :], cc_in[:, :])
                else:
                    nc.gpsimd.collective_compute(
                        "AllGather", ALU.bypass,
                        ins=[cc_in[:]], outs=[cc_out[:]],
                        replica_groups=[list(range(n_cores))],
                    )

            if phase == "full":
                nc.sync.dma_start(
                    azdst_all[:],
                    cc_in[:, 18:19].rearrange("(t p) c -> p t c", p=P),
                )
                edge_phase(2)
                # batched log_softmax over all dst tiles (2 act-table loads)
                with tc.tile_pool(name="fin", bufs=1) as fp:
                    mx = fp.tile([P, NT, 1], dt.float32)
                    nc.vector.tensor_reduce(
                        mx[:], o2_all[:], axis=mybir.AxisListType.X, op=ALU.max)
                    o2m = fp.tile([P, NT, 16], dt.float32)
                    nc.vector.tensor_tensor(
                        out=o2m[:], in0=o2_all[:],
                        in1=mx[:].to_broadcast([P, NT, 16]), op=ALU.subtract)
                    ex = fp.tile([P, NT, 16], dt.float32)
                    nc.scalar.activation(ex[:], o2m[:], AF.Exp)
                    ssum = fp.tile([P, NT, 1], dt.float32)
                    nc.vector.tensor_reduce(
                        ssum[:], ex[:], axis=mybir.AxisListType.X, op=ALU.add)
                    lse = fp.tile([P, NT, 1], dt.float32)
                    nc.scalar.activation(lse[:], ssum[:], AF.Ln)
                    res = fp.tile([P, NT, 16], dt.float32)
                    nc.vector.tensor_tensor(
                        out=res[:], in0=o2m[:],
                        in1=lse[:].to_broadcast([P, NT, 16]), op=ALU.subtract)
                    nc.sync.dma_start(
                        out_shard[:].rearrange("(t p) c -> p t c", p=P), res[:])

    nc.compile()
    return nc


# ----------------------------------------------------------------------------
# host entry
# ----------------------------------------------------------------------------

def _blockdiag(att, heads, hid):
    """[heads, hid] -> [heads*hid, heads] block diagonal."""
    out = np.zeros((heads * hid, max(heads, 1)), np.float32)
    for h in range(heads):
        out[h * hid:(h + 1) * hid, h] = att[h]
    return out


def prepare_inputs(inputs, sched: Schedule):
    x = np.asarray(inputs["x"], np.float32)
    ei = np.asarray(inputs["edge_index"])
    W1 = np.asarray(inputs["W1"], np.float32)
    as1 = np.asarray(inputs["att_src1"], np.float32)
    ad1 = np.asarray(inputs["att_dst1"], np.float32)
    W2 = np.asarray(inputs["W2"], np.float32)
    as2 = np.asarray(inputs["att_src2"], np.float32)
    ad2 = np.asarray(inputs["att_dst2"], np.float32)

    N, IN = x.shape
    TR = sched.table_rows
    xp = np.zeros((TR, IN), np.float32)
    xp[:N] = x
    F8 = ml_dtypes.float8_e4m3
    xTb = np.ascontiguousarray(
        xp.T.reshape(2, P, TR).transpose(1, 0, 2)).astype(F8)
    # interleave h columns: new col j = c*8 + h  <->  old col h*64 + c
    perm = np.array([(j % 8) * 64 + (j // 8) for j in range(512)], np.int64)
    acat = np.concatenate(
        [_blockdiag(as1, 8, 64), _blockdiag(ad1, 8, 64)], axis=1)  # [512, 16]
    wcat = W1 @ acat                                     # [256, 16]
    W1c = np.concatenate([W1[:, perm], wcat], axis=1)    # [256, 528]
    W1b = np.ascontiguousarray(
        (W1c * 16.0).reshape(2, P, 528).transpose(1, 0, 2)).astype(F8)
    att2b = np.concatenate([as2.T, ad2.T], axis=1)       # [16, 2]
    wzf = np.concatenate([W2[perm, :], W2[perm, :] @ att2b], axis=1)  # [512, 18]
    wzb = np.ascontiguousarray(wzf.reshape(4, P, 18).transpose(1, 0, 2)).astype(BF)

    shared = dict(xT=xTb, W1r=W1b, wzr=wzb)
    maps = []
    for c in range(sched.n_cores):
        pc = sched.per_core[c]
        m = dict(shared)
        m.update(
            idx1A=pc["idx1A"], idx1B=pc["idx1B"], idx2A=pc["idx2A"],
            idx2B=pc["idx2B"], dstloc=pc["dstloc"], dstlocT=pc["dstlocT"],
        )
        maps.append(m)
    return maps


_LAST_RESULT = {}


def kernel(**inputs):
    from concourse.bass_utils import run_bass_kernel_spmd

    x = np.asarray(inputs["x"], np.float32)
    ei = np.asarray(inputs["edge_index"], np.int64)
    N = x.shape[0]
    n_cores = 8
    loops = np.arange(N, dtype=np.int64)
    src = np.concatenate([ei[0], loops])
    dst = np.concatenate([ei[1], loops])

    sched = Schedule(src, dst, N, n_cores)
    phase = os.environ.get("GAT_PHASE", "full")
    nc = build_program(sched, n_cores, phase=phase)
    in_maps = prepare_inputs(inputs, sched)

    trace = bool(int(os.environ.get("GAT_TRACE", "0")))
    res = run_bass_kernel_spmd(
        nc, in_maps, core_ids=list(range(n_cores)), trace=trace,
    )
    _LAST_RESULT["res"] = res

    out = np.zeros((N, 16), np.float32)
    for c in range(n_cores):
        sh = res.results[c]["out_shard"]
        n0 = c * sched.npc
        out[n0:n0 + sched.npc] = sh[: sched.npc]
    return out

